# revision 4
# baseline (speedup 1.0000x reference)
"""Causal self-attention (B=4, T=2048, C=1024, H=16, rope) on 8 trn2 cores.

Sharding: data-parallel over B (4) x tensor-parallel over heads (2 groups of
8 heads). Core (b, g) computes its batch's Q/K/V for its 8 heads, the full
causal attention for those heads, and a partial output projection
(y_heads @ wp_cols.T). Host sums the two head-group partials per batch and
adds the output bias.

Device layout notes:
  - Q^T/K^T are kept as [c_out, t] tiles (partition = head-dim, 2 heads per
    128-partition tile) so QK^T needs no transposes; scores are computed as
    S^T[j, i] tiles (partition = key pos, free = query pos).
  - RoPE is applied as qt = ps*C + swap(ps)*S' where swap() is realized by
    32-row SBUF->SBUF DMAs on the sync queue and C/S' are host-precomputed
    tables.
  - Softmax denominators come free from an extra all-ones column appended to
    V (row 64 of the O^T accumulation); no max-subtraction is needed because
    the logits are bounded for this problem scale.
  - The causal mask for diagonal 128x128 blocks is applied AFTER the exp as
    a 0/1 tensor_mul on GpSimd (saves the PE mask-matmuls; exp of unmasked
    entries is bounded ~e^3 so no overflow).
  - The denominator chain is: DVE reciprocal straight off the O^T PSUM row,
    then a partition-broadcast DMA (stride-0 read) on the gpsimd queue, then
    a GpSimd norm-multiply -- no pack/unpack round trips.
  - HBM tensors are laid out so every initial DMA reads fully-contiguous
    per-partition lines (8-16 KiB descriptors, full HBM rate).
  - Matmul operands are bf16 (full-rate PE path); accumulation stays fp32 in
    PSUM. A short PE pre-warm covers the initial DMA window so the HAM clock
    gate is at 8/8 when real work arrives.
"""

import sys

if "/opt/trn_rl_repo" not in sys.path:
    sys.path.insert(0, "/opt/trn_rl_repo")

from contextlib import ExitStack

import numpy as np

import concourse.bass as bass
import concourse.mybir as mybir
from concourse import bacc
from concourse.bass_utils import run_bass_kernel_spmd
from concourse.tile import TileContext

B, T, C = 4, 2048, 1024
H = 16
D = 64
NCORES = 8
CL = C // 2  # per-core c_out (8 heads * 64)
HL = 8  # local heads
F = mybir.dt.float32
FR = mybir.dt.bfloat16  # matmul operand dtype

_NC_CACHE = {}


def _build_nc(with_bias: bool):
    KC = 9 if with_bias else 8  # c_in chunks of 128 (one extra for bias row)
    nc = bacc.Bacc("TRN2", debug=False, num_devices=NCORES)

    # Host-side pre-swizzled layouts: fully-contiguous per-partition lines so
    # each initial DMA moves 8-16KB per partition in one descriptor.
    xT = nc.declare_dram_parameter("xT", [128, 2, KC * 1024], FR, isOutput=False).ap()
    wqT = nc.declare_dram_parameter("wqT", [128, KC * CL], FR, isOutput=False).ap()
    wkT = nc.declare_dram_parameter("wkT", [128, KC * CL], FR, isOutput=False).ap()
    wvT = nc.declare_dram_parameter("wvT", [128, KC * CL], FR, isOutput=False).ap()
    wpT = nc.declare_dram_parameter("wpT", [128, 4 * C], FR, isOutput=False).ap()
    ropeC = nc.declare_dram_parameter("ropeC", [128, T], FR, isOutput=False).ap()
    ropeS = nc.declare_dram_parameter("ropeS", [128, T], FR, isOutput=False).ap()
    mask01 = nc.declare_dram_parameter("mask01", [128, 128], FR, isOutput=False).ap()
    out = nc.declare_dram_parameter("out", [T, C], F, isOutput=True).ap()

    EXP = mybir.ActivationFunctionType.Exp
    scale = 1.0 / float(np.sqrt(D))

    with TileContext(nc) as tc:
        with ExitStack() as ctx:
            # pools that live across both phases
            qk_pool = ctx.enter_context(tc.tile_pool(name="qk", bufs=1))
            v_pool = ctx.enter_context(tc.tile_pool(name="v", bufs=1))
            c2 = ctx.enter_context(tc.tile_pool(name="c2", bufs=1))

            qt_sb = [
                qk_pool.tile([128, T], FR, tag=f"qt{m}", name=f"qt{m}")
                for m in range(4)
            ]
            kt_sb = [
                qk_pool.tile([128, T], FR, tag=f"kt{m}", name=f"kt{m}")
                for m in range(4)
            ]
            # all 16 V tiles in one buffer: [t-block, head, D+1]; col D is the
            # all-ones denominator column
            vaug = v_pool.tile([128, 16, HL, D + 1], FR, tag="va", name="va")

            wp_sb = c2.tile([128, 4 * C], FR, tag="wp", name="wp")
            mk_sb = c2.tile([128, 128], FR, tag="mk", name="mk")

            # ---------------- phase 1: QKV projections + rope ----------------
            with ExitStack() as p1:
                wpool = p1.enter_context(tc.tile_pool(name="w", bufs=1))
                xpool = p1.enter_context(tc.tile_pool(name="x", bufs=1))
                rpool = p1.enter_context(tc.tile_pool(name="rope", bufs=1))
                tpool = p1.enter_context(tc.tile_pool(name="t1", bufs=3))
                ps1 = p1.enter_context(tc.tile_pool(name="ps1", bufs=4, space="PSUM"))
                wrm = p1.enter_context(tc.tile_pool(name="wrm", bufs=1, space="PSUM"))

                # PE pre-warm: keep the HAM clock gate at 8/8 while the input
                # DMAs land, so the first real matmuls run at 2.4 GHz.
                warm_sb = tpool.tile([128, 512], FR, tag="warm", name="warm")
                nc.vector.memset(warm_sb, 0.0)
                warm_ps = wrm.tile([128, 512], F, tag="wps", name="wps")
                for _ in range(18):
                    nc.tensor.matmul(
                        warm_ps,
                        lhsT=warm_sb[:, 0:128],
                        rhs=warm_sb,
                        start=True,
                        stop=True,
                        skip_group_check=True,
                    )

                # initial loads: x halves on the sync queue, weights on the
                # scalar queue -- all transfers are fully contiguous per
                # partition so they run at HBM line rate. The first K matmul
                # needs only wk + x half 0 (~3MB).
                x_sb = xpool.tile([128, 2, KC * 1024], FR, tag="x", name="x")
                wk_sb = wpool.tile([128, KC * CL], FR, tag="wk", name="wk")
                wq_sb = wpool.tile([128, KC * CL], FR, tag="wq", name="wq")
                wv_sb = wpool.tile([128, KC * CL], FR, tag="wv", name="wv")
                nc.scalar.dma_start(out=wk_sb, in_=wkT)
                nc.sync.dma_start(out=x_sb[:, 0, :], in_=xT[:, 0, :])
                nc.scalar.dma_start(out=wq_sb, in_=wqT)
                nc.scalar.dma_start(out=x_sb[:, 1, :], in_=xT[:, 1, :])
                nc.scalar.dma_start(out=wv_sb, in_=wvT)
                nc.scalar.dma_start(out=wp_sb, in_=wpT)

                rc_sb = rpool.tile([128, T], FR, tag="rc", name="rc")
                rs_sb = rpool.tile([128, T], FR, tag="rs", name="rs")
                nc.gpsimd.dma_start(out=rc_sb, in_=ropeC)
                nc.gpsimd.dma_start(out=rs_sb, in_=ropeS)
                nc.gpsimd.dma_start(out=mk_sb, in_=mask01)

                # ones column of vaug (denominator trick)
                nc.vector.memset(vaug[:, :, :, D : D + 1], 1.0)

                def x_rhs(k, lo, hi):
                    """x^T slice [128, hi-lo] for contraction chunk k, T cols
                    [lo, hi) (must stay within one T-half)."""
                    h = lo // 1024
                    o = lo - 1024 * h
                    return x_sb[:, h, 1024 * k + o : 1024 * k + o + (hi - lo)]

                def emit_kq(m, t):
                    # K and Q tiles ([c_out, t] layout) + rope, paired per
                    # (m, t) so each 32-row rope swap DMA moves both
                    # projections at once (2KB per-partition lines). Swap DMAs
                    # ride the sync queue, which is idle after the x load.
                    kq_ps = []
                    for wsb in (wk_sb, wq_sb):
                        ps = ps1.tile([128, 512], F, tag="ps", name="ps")
                        for k in range(KC):
                            nc.tensor.matmul(
                                ps,
                                lhsT=wsb[:, CL * k + 128 * m : CL * k + 128 * (m + 1)],
                                rhs=x_rhs(k, 512 * t, 512 * (t + 1)),
                                start=(k == 0),
                                stop=(k == KC - 1),
                            )
                        kq_ps.append(ps)
                    cp = tpool.tile([128, 1024], FR, tag="cp", name="cp")
                    nc.scalar.copy(cp[:, 0:512], kq_ps[0])
                    nc.scalar.copy(cp[:, 512:1024], kq_ps[1])
                    sw = tpool.tile([128, 1024], FR, tag="sw", name="sw")
                    for a, b in ((0, 32), (32, 0), (64, 96), (96, 64)):
                        nc.sync.dma_start(out=sw[a : a + 32, :], in_=cp[b : b + 32, :])
                    ts = slice(512 * t, 512 * (t + 1))
                    t1 = tpool.tile([128, 1024], FR, tag="t1", name="t1")
                    nc.vector.tensor_mul(t1[:, 0:512], kq_ps[0], rc_sb[:, ts])
                    nc.vector.tensor_mul(t1[:, 512:1024], kq_ps[1], rc_sb[:, ts])
                    t2 = tpool.tile([128, 1024], FR, tag="t2", name="t2")
                    nc.gpsimd.tensor_mul(t2[:, 0:512], sw[:, 0:512], rs_sb[:, ts])
                    nc.gpsimd.tensor_mul(t2[:, 512:1024], sw[:, 512:1024], rs_sb[:, ts])
                    nc.vector.tensor_add(kt_sb[m][:, ts], t1[:, 0:512], t2[:, 0:512])
                    nc.vector.tensor_add(
                        qt_sb[m][:, ts], t1[:, 512:1024], t2[:, 512:1024]
                    )

                def emit_v(jj):
                    # V tile (natural [t, c_out] layout) -> vaug, extraction on
                    # the scalar engine
                    ps = ps1.tile([128, 512], F, tag="ps", name="ps")
                    for k in range(KC):
                        nc.tensor.matmul(
                            ps,
                            lhsT=x_rhs(k, 128 * jj, 128 * (jj + 1)),
                            rhs=wv_sb[:, CL * k : CL * (k + 1)],
                            start=(k == 0),
                            stop=(k == KC - 1),
                        )
                    nc.scalar.copy(
                        out=vaug[:, jj, :, 0:D],
                        in_=ps.rearrange("p (h d) -> p h d", h=HL),
                    )

                # interleave: K/Q for the first T-half, then V tiles of that
                # half (so attention for early ci can start while the second
                # half projects), then the rest.
                for m in range(4):
                    for t in (0, 1):
                        emit_kq(m, t)
                for jj in range(8):
                    emit_v(jj)
                for m in range(4):
                    for t in (2, 3):
                        emit_kq(m, t)
                for jj in range(8, 16):
                    emit_v(jj)

            # ---------------- phase 2: attention + output projection ---------
            ppool = ctx.enter_context(tc.tile_pool(name="pt", bufs=3))
            yrawp = ctx.enter_context(tc.tile_pool(name="yraw", bufs=4))
            ynp = ctx.enter_context(tc.tile_pool(name="yn", bufs=9))
            osbp = ctx.enter_context(tc.tile_pool(name="osb", bufs=3))
            dpool = ctx.enter_context(tc.tile_pool(name="dd", bufs=2))
            bcpool = ctx.enter_context(tc.tile_pool(name="bc", bufs=3))
            spool = ctx.enter_context(tc.tile_pool(name="sps", bufs=2, space="PSUM"))
            opool = ctx.enter_context(tc.tile_pool(name="ops", bufs=4, space="PSUM"))

            def emit_outproj_chunk(ci, yn, chunk):
                for g in (2 * chunk, 2 * chunk + 1):
                    tt, cc = g % 4, g // 4
                    pr = opool.tile([128, 512], F, tag="o", name="pr")
                    for p in range(4):
                        nc.tensor.matmul(
                            pr,
                            lhsT=yn[p][:, 128 * tt : 128 * (tt + 1)],
                            rhs=wp_sb[:, 1024 * p + 512 * cc : 1024 * p + 512 * (cc + 1)],
                            start=(p == 0),
                            stop=(p == 3),
                        )
                    osb = osbp.tile([128, 512], F, tag="osb", name="osb")
                    nc.vector.tensor_copy(osb, pr)
                    nc.sync.dma_start(
                        out=out[
                            512 * ci + 128 * tt : 512 * ci + 128 * (tt + 1),
                            512 * cc : 512 * (cc + 1),
                        ],
                        in_=osb,
                    )

            outq = []  # (yn_list, ci, next_chunk) FIFO of outproj work
            dve_q = []  # deferred boundary ops, drained 2 per tj iter
            pending_norm = None
            for ci in range(4):
                yn = []
                for p in range(4):
                    # bound the deferral window to one boundary's worth so
                    # pool reuse (WAR) tracking stays consistent with the
                    # actual emission order
                    while len(dve_q) > 7:
                        dve_q.pop(0)()
                    o_ps = [
                        opool.tile([128, 512], F, tag="o", name="o") for _ in range(2)
                    ]
                    ntj = 4 * ci + 4
                    for tj in range(ntj):
                        kk = tj - 4 * ci
                        off = 128 * max(kk, 0)
                        s_ps = spool.tile([128, 1024], F, tag="s", name="s")
                        for h in range(2):
                            nc.tensor.matmul(
                                s_ps[:, 512 * h + off : 512 * h + 512],
                                lhsT=kt_sb[p][
                                    64 * h : 64 * h + 64,
                                    128 * tj : 128 * (tj + 1),
                                ],
                                rhs=qt_sb[p][
                                    64 * h : 64 * h + 64,
                                    512 * ci + off : 512 * (ci + 1),
                                ],
                                start=True,
                                stop=True,
                                tile_position=(64 * h, 0),
                                skip_group_check=True,
                            )
                        pt = ppool.tile([128, 1024], FR, tag="pt", name="pt")
                        if kk < 0:
                            nc.scalar.activation(pt, s_ps, EXP, scale=scale)
                        else:
                            s_v = s_ps.rearrange("q (h n) -> q h n", h=2)[:, :, off:]
                            p_v = pt.rearrange("q (h n) -> q h n", h=2)[:, :, off:]
                            nc.scalar.activation(p_v, s_v, EXP, scale=scale)
                            # causal mask for the diagonal 128x128 block:
                            # multiply by the 0/1 upper-tri mask (split across
                            # DVE and GpSimd; keeps the PE out of the mask
                            # business)
                            for h, eng in ((0, nc.vector), (1, nc.gpsimd)):
                                eng.tensor_mul(
                                    pt[:, 512 * h + off : 512 * h + off + 128],
                                    pt[:, 512 * h + off : 512 * h + off + 128],
                                    mk_sb,
                                )
                        for h in range(2):
                            nc.tensor.matmul(
                                o_ps[h][0 : D + 1, off:512],
                                lhsT=vaug[:, tj, 2 * p + h, :],
                                rhs=pt[:, 512 * h + off : 512 * h + 512],
                                start=(tj == 0),
                                stop=(tj == ntj - 1),
                                skip_group_check=True,
                            )
                        for _ in range(2):
                            if dve_q:
                                dve_q.pop(0)()
                    # epilogue, ordered so the recip -> bc-broadcast ->
                    # norm-mul chain is always a full boundary ahead of its
                    # consumer and the PE never waits on it:
                    #   1. reciprocal of the denominator rows straight out of
                    #      PSUM (row 64 of each head's O^T accumulation)
                    #   2. compact both heads' O (shifted DVE copies, no DMA)
                    #   3. partition-broadcast of the reciprocals (stride-0
                    #      gpsimd DMA, feeds NEXT boundary's deferred norm)
                    #   4. deferred norm for the previous pair (GpSimd mul)
                    #   5. one outproj chunk-call from the FIFO (two
                    #      boundaries behind its ci, so its yn lhsT is ready)
                    yraw = yrawp.tile([128, 512], F, tag="yraw", name="yraw")
                    d_sb = dpool.tile([128, 512], F, tag="D", name="D")
                    dve_q.append(
                        lambda d=d_sb, o=o_ps[0]: nc.vector.reciprocal(
                            d[0:1, 0:512], o[64:65, :]
                        )
                    )
                    dve_q.append(
                        lambda y=yraw, o=o_ps[0]: nc.vector.tensor_copy(
                            y[0:64, :], o[0:64, :]
                        )
                    )
                    dve_q.append(
                        lambda d=d_sb, o=o_ps[1]: nc.vector.reciprocal(
                            d[64:65, 0:512], o[64:65, :]
                        )
                    )
                    dve_q.append(
                        lambda y=yraw, o=o_ps[1]: nc.vector.tensor_copy(
                            y[64:128, :], o[0:64, :]
                        )
                    )
                    bc = bcpool.tile([128, 512], F, tag="bc", name="bc")

                    def mk_bc(d_sb, bc, h):
                        def f():
                            sl = d_sb[64 * h : 64 * h + 1, 0:512]
                            bsrc = bass.AP(
                                tensor=sl.tensor,
                                offset=sl.offset,
                                ap=[list(sl.ap[0]), [0, 64], [1, 512]],
                            )
                            nc.gpsimd.dma_start(
                                out=bc[64 * h : 64 * h + 64, :], in_=bsrc
                            )

                        return f

                    for h in range(2):
                        dve_q.append(mk_bc(d_sb, bc, h))

                    def mk_norm(pyn, pyraw, pbc):
                        def f():
                            pynorm = ynp.tile([128, 512], FR, tag="yn", name="yn")
                            nc.gpsimd.tensor_mul(pynorm, pyraw, pbc)
                            pyn.append(pynorm)

                        return f

                    if pending_norm is not None:
                        dve_q.append(mk_norm(*pending_norm))
                        pending_norm = None
                    if ci == 3:
                        dve_q.append(mk_norm(yn, yraw, bc))
                    else:
                        pending_norm = (yn, yraw, bc)
                    ncalls = 2 if len(outq) > 1 else 1
                    for _ in range(ncalls):
                        if not (
                            outq
                            and outq[0][1] <= ci - 1
                            and (p >= 1 or outq[0][1] <= ci - 2)
                        ):
                            break
                        pyn_l, pci, chunk = outq[0]
                        while len(pyn_l) < 4 and dve_q:
                            dve_q.pop(0)()
                        emit_outproj_chunk(pci, pyn_l, chunk)
                        if chunk == 3:
                            outq.pop(0)
                        else:
                            outq[0][2] = chunk + 1

                outq.append([yn, ci, 0])
            for pyn_l, pci, chunk in [
                (q[0], q[1], c) for q in outq for c in range(q[2], 4)
            ]:
                while len(pyn_l) < 4 and dve_q:
                    dve_q.pop(0)()
                emit_outproj_chunk(pci, pyn_l, chunk)
            while dve_q:
                dve_q.pop(0)()

    nc.compile()
    return nc


def _get_nc(with_bias: bool):
    if with_bias not in _NC_CACHE:
        _NC_CACHE[with_bias] = _build_nc(with_bias)
    return _NC_CACHE[with_bias]


def _rope_tables():
    half = D // 2
    i = np.arange(half, dtype=np.float32)
    expo = (2.0 * i / np.float32(D)).astype(np.float32)
    alpha = (1.0 / (np.float32(10000.0) ** expo)).astype(np.float32)
    ang = (np.arange(T, dtype=np.float32)[:, None] * alpha[None, :]).astype(np.float32)
    cosv = np.cos(ang).astype(np.float32).T  # [32, T]
    sinv = np.sin(ang).astype(np.float32).T
    c64 = np.concatenate([cosv, cosv], axis=0)  # [64, T]
    s64 = np.concatenate([-sinv, sinv], axis=0)
    ropeC = np.ascontiguousarray(np.concatenate([c64, c64], axis=0))  # [128, T]
    ropeS = np.ascontiguousarray(np.concatenate([s64, s64], axis=0))
    import ml_dtypes

    return ropeC.astype(ml_dtypes.bfloat16), ropeS.astype(ml_dtypes.bfloat16)


import ml_dtypes


def _round_fp32r(a):
    """Cast host data to the matmul operand dtype (bf16)."""
    return np.ascontiguousarray(
        np.asarray(a, dtype=np.float32).astype(ml_dtypes.bfloat16)
    )


def _swizzle_w(wT):
    """[KC*128, M] -> [128, KC*M] with fully-contiguous per-partition lines."""
    kc = wT.shape[0] // 128
    m = wT.shape[1]
    return np.ascontiguousarray(
        wT.reshape(kc, 128, m).transpose(1, 0, 2).reshape(128, kc * m)
    )


def _swizzle_x(xb):
    """[KC*128, T] -> [128, 2, KC*1024]: partition-major, T-half-major, then
    (chunk, within-half-col) contiguous."""
    kc = xb.shape[0] // 128
    return np.ascontiguousarray(
        xb.reshape(kc, 128, 2, 1024).transpose(1, 2, 0, 3).reshape(128, 2, kc * 1024)
    )


def _make_in_maps(x, wq, bq, wk, bk, wv, bv, wp, with_bias):
    ropeC, ropeS = _rope_tables()
    # 0/1 causal keep-mask for the diagonal 128x128 block: 1 where j <= i
    # (keep: key j, query i), 0 where masked
    mask01 = np.triu(np.ones((128, 128), np.float32)).astype(ml_dtypes.bfloat16)
    in_maps = []
    for b in range(B):
        xb = np.ascontiguousarray(x[b].T.astype(np.float32, copy=False))  # [C, T]
        if with_bias:
            aug = np.zeros((9 * 128 - C, T), np.float32)
            aug[0, :] = 1.0
            xb = np.concatenate([xb, aug], axis=0)
        xbs = _swizzle_x(_round_fp32r(xb))
        for g in range(2):
            sl = slice(g * CL, (g + 1) * CL)
            wqTc = np.ascontiguousarray(wq[sl, :].T.astype(np.float32, copy=False))
            wkTc = np.ascontiguousarray(wk[sl, :].T.astype(np.float32, copy=False))
            wvTc = np.ascontiguousarray(wv[sl, :].T.astype(np.float32, copy=False))
            if with_bias:
                npad = 9 * 128 - C

                def _aug_w(wT, bias):
                    a = np.zeros((npad, CL), np.float32)
                    a[0, :] = bias[sl].astype(np.float32, copy=False)
                    return np.ascontiguousarray(np.concatenate([wT, a], axis=0))

                wqTc = _aug_w(wqTc, bq)
                wkTc = _aug_w(wkTc, bk)
                wvTc = _aug_w(wvTc, bv)
            wpTc = np.ascontiguousarray(wp[:, sl].T.astype(np.float32, copy=False))
            in_maps.append(
                {
                    "xT": xbs,
                    "wqT": _swizzle_w(_round_fp32r(wqTc)),
                    "wkT": _swizzle_w(_round_fp32r(wkTc)),
                    "wvT": _swizzle_w(_round_fp32r(wvTc)),
                    "wpT": _swizzle_w(_round_fp32r(wpTc)),
                    "ropeC": ropeC,
                    "ropeS": ropeS,
                    "mask01": mask01,
                }
            )
    return in_maps


def _gather(results, bp):
    out = np.empty((B, T, C), dtype=np.float32)
    bp32 = np.asarray(bp, dtype=np.float32)
    for b in range(B):
        out[b] = results[2 * b]["out"] + results[2 * b + 1]["out"] + bp32
    return out


def run(x, wq, bq, wk, bk, wv, bv, wp, bp, trace=False, **kw):
    """Build/compile (cached), run on 8 cores, gather. Returns (out, results)."""
    arrs = [np.asarray(a) for a in (x, wq, bq, wk, bk, wv, bv, wp, bp)]
    x, wq, bq, wk, bk, wv, bv, wp, bp = arrs
    with_bias = bool(np.any(bq) or np.any(bk) or np.any(bv))
    nc = _get_nc(with_bias)
    in_maps = _make_in_maps(x, wq, bq, wk, bk, wv, bv, wp, with_bias)
    res = run_bass_kernel_spmd(nc, in_maps, list(range(NCORES)), trace=trace, **kw)
    return _gather(res.results, bp), res


def kernel(x, wq, bq, wk, bk, wv, bv, wp, bp):
    out, _ = run(x, wq, bq, wk, bk, wv, bv, wp, bp)
    return out


# revision 8
# speedup vs baseline: 1.2492x; 1.2492x over previous
"""Causal self-attention (B=4, T=2048, C=1024, H=16, rope) on 8 trn2 cores.

Sharding: data-parallel over B (4) x tensor-parallel over heads (2 groups of
8 heads). Core (b, g) computes its batch's Q/K/V for its 8 heads, the full
causal attention for those heads, and a partial output projection
(y_heads @ wp_cols.T). Host sums the two head-group partials per batch and
adds the output bias.

Device layout notes:
  - Q^T/K^T are kept as [c_out, t] tiles (partition = head-dim, 2 heads per
    128-partition tile) so QK^T needs no transposes; scores are computed as
    S^T[j, i] tiles (partition = key pos, free = query pos).
  - RoPE is applied as qt = ps*C + swap(ps)*S' where swap() is realized by
    32-row SBUF->SBUF DMAs on the sync queue and C/S' are host-precomputed
    tables.
  - Softmax denominators come free from an extra all-ones column appended to
    V (row 64 of the O^T accumulation); no max-subtraction is needed because
    the logits are bounded for this problem scale.
  - The causal mask for diagonal 128x128 blocks is applied AFTER the exp as
    a 0/1 tensor_mul on GpSimd (saves the PE mask-matmuls; exp of unmasked
    entries is bounded ~e^3 so no overflow).
  - The denominator chain is: DVE reciprocal straight off the O^T PSUM row,
    then a partition-broadcast DMA (stride-0 read) on the gpsimd queue, then
    a GpSimd norm-multiply -- no pack/unpack round trips.
  - HBM tensors are laid out so every initial DMA reads fully-contiguous
    per-partition lines (8-16 KiB descriptors, full HBM rate).
  - Matmul operands are bf16 (full-rate PE path); accumulation stays fp32 in
    PSUM. A short PE pre-warm covers the initial DMA window so the HAM clock
    gate is at 8/8 when real work arrives.
"""

import sys

if "/opt/trn_rl_repo" not in sys.path:
    sys.path.insert(0, "/opt/trn_rl_repo")

from contextlib import ExitStack

import numpy as np

import concourse.bass as bass
import concourse.mybir as mybir
from concourse import bacc
from concourse.bass_utils import run_bass_kernel_spmd
from concourse.tile import TileContext

B, T, C = 4, 2048, 1024
H = 16
D = 64
NCORES = 8
CL = C // 2  # per-core c_out (8 heads * 64)
HL = 8  # local heads
F = mybir.dt.float32
FR = mybir.dt.bfloat16  # matmul operand dtype

_NC_CACHE = {}


def _build_nc(with_bias: bool):
    KC = 9 if with_bias else 8  # c_in chunks of 128 (one extra for bias row)
    nc = bacc.Bacc("TRN2", debug=False, num_devices=NCORES)

    # Host-side pre-swizzled layouts: fully-contiguous per-partition lines so
    # each initial DMA moves 8-16KB per partition in one descriptor.
    xT = nc.declare_dram_parameter("xT", [128, 2, KC * 1024], FR, isOutput=False).ap()
    wqT = nc.declare_dram_parameter("wqT", [128, KC * CL], FR, isOutput=False).ap()
    wkT = nc.declare_dram_parameter("wkT", [128, KC * CL], FR, isOutput=False).ap()
    wvT = nc.declare_dram_parameter("wvT", [128, KC * CL], FR, isOutput=False).ap()
    wpT = nc.declare_dram_parameter("wpT", [128, 4 * C], FR, isOutput=False).ap()
    ropeC = nc.declare_dram_parameter("ropeC", [128, T], FR, isOutput=False).ap()
    ropeS = nc.declare_dram_parameter("ropeS", [128, T], FR, isOutput=False).ap()
    mask01 = nc.declare_dram_parameter("mask01", [128, 128], FR, isOutput=False).ap()
    out = nc.declare_dram_parameter("out", [T, C], F, isOutput=True).ap()

    EXP = mybir.ActivationFunctionType.Exp
    scale = 1.0 / float(np.sqrt(D))

    with TileContext(nc) as tc:
        with ExitStack() as ctx:
            # pools that live across both phases
            qk_pool = ctx.enter_context(tc.tile_pool(name="qk", bufs=1))
            v_pool = ctx.enter_context(tc.tile_pool(name="v", bufs=1))
            c2 = ctx.enter_context(tc.tile_pool(name="c2", bufs=1))

            qt_sb = [
                qk_pool.tile([128, T], FR, tag=f"qt{m}", name=f"qt{m}")
                for m in range(4)
            ]
            kt_sb = [
                qk_pool.tile([128, T], FR, tag=f"kt{m}", name=f"kt{m}")
                for m in range(4)
            ]
            # all 16 V tiles in one buffer: [t-block, head, D+1]; col D is the
            # all-ones denominator column
            vaug = v_pool.tile([128, 16, HL, D + 1], FR, tag="va", name="va")

            wp_sb = c2.tile([128, 4 * C], FR, tag="wp", name="wp")
            mk_sb = c2.tile([128, 128], FR, tag="mk", name="mk")

            # ---------------- phase 1: QKV projections + rope ----------------
            with ExitStack() as p1:
                wpool = p1.enter_context(tc.tile_pool(name="w", bufs=1))
                xpool = p1.enter_context(tc.tile_pool(name="x", bufs=1))
                rpool = p1.enter_context(tc.tile_pool(name="rope", bufs=1))
                tpool = p1.enter_context(tc.tile_pool(name="t1", bufs=3))
                ps1 = p1.enter_context(tc.tile_pool(name="ps1", bufs=4, space="PSUM"))
                wrm = p1.enter_context(tc.tile_pool(name="wrm", bufs=1, space="PSUM"))

                # PE pre-warm: keep the HAM clock gate at 8/8 while the input
                # DMAs land, so the first real matmuls run at 2.4 GHz.
                warm_sb = tpool.tile([128, 512], FR, tag="warm", name="warm")
                nc.vector.memset(warm_sb, 0.0)
                warm_ps = wrm.tile([128, 512], F, tag="wps", name="wps")
                for _ in range(18):
                    nc.tensor.matmul(
                        warm_ps,
                        lhsT=warm_sb[:, 0:128],
                        rhs=warm_sb,
                        start=True,
                        stop=True,
                        skip_group_check=True,
                    )

                # initial loads: x halves on the sync queue, weights on the
                # scalar queue -- all transfers are fully contiguous per
                # partition so they run at HBM line rate. The first K matmul
                # needs only wk + x half 0 (~3MB).
                x_sb = xpool.tile([128, 2, KC * 1024], FR, tag="x", name="x")
                wk_sb = wpool.tile([128, KC * CL], FR, tag="wk", name="wk")
                wq_sb = wpool.tile([128, KC * CL], FR, tag="wq", name="wq")
                wv_sb = wpool.tile([128, KC * CL], FR, tag="wv", name="wv")
                nc.scalar.dma_start(out=wk_sb, in_=wkT)
                nc.sync.dma_start(out=x_sb[:, 0, :], in_=xT[:, 0, :])
                nc.scalar.dma_start(out=wq_sb, in_=wqT)
                nc.scalar.dma_start(out=x_sb[:, 1, :], in_=xT[:, 1, :])
                nc.scalar.dma_start(out=wv_sb, in_=wvT)
                nc.scalar.dma_start(out=wp_sb, in_=wpT)

                rc_sb = rpool.tile([128, T], FR, tag="rc", name="rc")
                rs_sb = rpool.tile([128, T], FR, tag="rs", name="rs")
                nc.gpsimd.dma_start(out=rc_sb, in_=ropeC)
                nc.gpsimd.dma_start(out=rs_sb, in_=ropeS)
                nc.gpsimd.dma_start(out=mk_sb, in_=mask01)

                # ones column of vaug (denominator trick)
                nc.vector.memset(vaug[:, :, :, D : D + 1], 1.0)

                def x_rhs(k, lo, hi):
                    """x^T slice [128, hi-lo] for contraction chunk k, T cols
                    [lo, hi) (must stay within one T-half)."""
                    h = lo // 1024
                    o = lo - 1024 * h
                    return x_sb[:, h, 1024 * k + o : 1024 * k + o + (hi - lo)]

                def emit_kq(m, t):
                    # K and Q tiles ([c_out, t] layout) + rope, paired per
                    # (m, t) so each 32-row rope swap DMA moves both
                    # projections at once (2KB per-partition lines). Swap DMAs
                    # ride the sync queue, which is idle after the x load.
                    kq_ps = []
                    for wsb in (wk_sb, wq_sb):
                        ps = ps1.tile([128, 512], F, tag="ps", name="ps")
                        for k in range(KC):
                            nc.tensor.matmul(
                                ps,
                                lhsT=wsb[:, CL * k + 128 * m : CL * k + 128 * (m + 1)],
                                rhs=x_rhs(k, 512 * t, 512 * (t + 1)),
                                start=(k == 0),
                                stop=(k == KC - 1),
                            )
                        kq_ps.append(ps)
                    cp = tpool.tile([128, 1024], FR, tag="cp", name="cp")
                    nc.scalar.copy(cp[:, 0:512], kq_ps[0])
                    nc.scalar.copy(cp[:, 512:1024], kq_ps[1])
                    sw = tpool.tile([128, 1024], FR, tag="sw", name="sw")
                    for a, b in ((0, 32), (32, 0), (64, 96), (96, 64)):
                        nc.sync.dma_start(out=sw[a : a + 32, :], in_=cp[b : b + 32, :])
                    ts = slice(512 * t, 512 * (t + 1))
                    t1 = tpool.tile([128, 1024], FR, tag="t1", name="t1")
                    nc.vector.tensor_mul(t1[:, 0:512], kq_ps[0], rc_sb[:, ts])
                    nc.vector.tensor_mul(t1[:, 512:1024], kq_ps[1], rc_sb[:, ts])
                    t2 = tpool.tile([128, 1024], FR, tag="t2", name="t2")
                    nc.gpsimd.tensor_mul(t2[:, 0:512], sw[:, 0:512], rs_sb[:, ts])
                    nc.gpsimd.tensor_mul(t2[:, 512:1024], sw[:, 512:1024], rs_sb[:, ts])
                    nc.vector.tensor_add(kt_sb[m][:, ts], t1[:, 0:512], t2[:, 0:512])
                    nc.vector.tensor_add(
                        qt_sb[m][:, ts], t1[:, 512:1024], t2[:, 512:1024]
                    )

                def emit_v(jj):
                    # V tile (natural [t, c_out] layout) -> vaug, extraction on
                    # the scalar engine
                    ps = ps1.tile([128, 512], F, tag="ps", name="ps")
                    for k in range(KC):
                        nc.tensor.matmul(
                            ps,
                            lhsT=x_rhs(k, 128 * jj, 128 * (jj + 1)),
                            rhs=wv_sb[:, CL * k : CL * (k + 1)],
                            start=(k == 0),
                            stop=(k == KC - 1),
                        )
                    nc.scalar.copy(
                        out=vaug[:, jj, :, 0:D],
                        in_=ps.rearrange("p (h d) -> p h d", h=HL),
                    )

                # interleave: K/Q for the first T-half, then V tiles of that
                # half (so attention for early ci can start while the second
                # half projects), then the rest.
                for m in range(4):
                    for t in (0, 1):
                        emit_kq(m, t)
                for jj in range(8):
                    emit_v(jj)
                for m in range(4):
                    for t in (2, 3):
                        emit_kq(m, t)
                for jj in range(8, 16):
                    emit_v(jj)

            # ---------------- phase 2: attention + output projection ---------
            ppool = ctx.enter_context(tc.tile_pool(name="pt", bufs=3))
            yrawp = ctx.enter_context(tc.tile_pool(name="yraw", bufs=4))
            ynp = ctx.enter_context(tc.tile_pool(name="yn", bufs=9))
            osbp = ctx.enter_context(tc.tile_pool(name="osb", bufs=3))
            dpool = ctx.enter_context(tc.tile_pool(name="dd", bufs=2))
            bcpool = ctx.enter_context(tc.tile_pool(name="bc", bufs=3))
            spool = ctx.enter_context(tc.tile_pool(name="sps", bufs=2, space="PSUM"))
            opool = ctx.enter_context(tc.tile_pool(name="ops", bufs=4, space="PSUM"))

            def emit_outproj_chunk(ci, yn, chunk):
                for g in (2 * chunk, 2 * chunk + 1):
                    tt, cc = g % 4, g // 4
                    pr = opool.tile([128, 512], F, tag="o", name="pr")
                    for p in range(4):
                        nc.tensor.matmul(
                            pr,
                            lhsT=yn[p][:, 128 * tt : 128 * (tt + 1)],
                            rhs=wp_sb[:, 1024 * p + 512 * cc : 1024 * p + 512 * (cc + 1)],
                            start=(p == 0),
                            stop=(p == 3),
                        )
                    osb = osbp.tile([128, 512], F, tag="osb", name="osb")
                    nc.vector.tensor_copy(osb, pr)
                    nc.sync.dma_start(
                        out=out[
                            512 * ci + 128 * tt : 512 * ci + 128 * (tt + 1),
                            512 * cc : 512 * (cc + 1),
                        ],
                        in_=osb,
                    )

            outq = []  # (yn_list, ci, next_chunk) FIFO of outproj work
            dve_q = []  # deferred boundary ops, drained 2 per tj iter
            pending_norm = None
            for ci in range(4):
                yn = []
                for p in range(4):
                    # bound the deferral window to one boundary's worth so
                    # pool reuse (WAR) tracking stays consistent with the
                    # actual emission order
                    while len(dve_q) > 7:
                        dve_q.pop(0)()
                    o_ps = [
                        opool.tile([128, 512], F, tag="o", name="o") for _ in range(2)
                    ]
                    ntj = 4 * ci + 4
                    for tj in range(ntj):
                        kk = tj - 4 * ci
                        off = 128 * max(kk, 0)
                        s_ps = spool.tile([128, 1024], F, tag="s", name="s")
                        for h in range(2):
                            nc.tensor.matmul(
                                s_ps[:, 512 * h + off : 512 * h + 512],
                                lhsT=kt_sb[p][
                                    64 * h : 64 * h + 64,
                                    128 * tj : 128 * (tj + 1),
                                ],
                                rhs=qt_sb[p][
                                    64 * h : 64 * h + 64,
                                    512 * ci + off : 512 * (ci + 1),
                                ],
                                start=True,
                                stop=True,
                                tile_position=(64 * h, 0),
                                skip_group_check=True,
                            )
                        pt = ppool.tile([128, 1024], FR, tag="pt", name="pt")
                        if kk < 0:
                            nc.scalar.activation(pt, s_ps, EXP, scale=scale)
                        else:
                            s_v = s_ps.rearrange("q (h n) -> q h n", h=2)[:, :, off:]
                            p_v = pt.rearrange("q (h n) -> q h n", h=2)[:, :, off:]
                            nc.scalar.activation(p_v, s_v, EXP, scale=scale)
                            # causal mask for the diagonal 128x128 block:
                            # multiply by the 0/1 upper-tri mask (split across
                            # DVE and GpSimd; keeps the PE out of the mask
                            # business)
                            for h, eng in ((0, nc.vector), (1, nc.gpsimd)):
                                eng.tensor_mul(
                                    pt[:, 512 * h + off : 512 * h + off + 128],
                                    pt[:, 512 * h + off : 512 * h + off + 128],
                                    mk_sb,
                                )
                        for h in range(2):
                            nc.tensor.matmul(
                                o_ps[h][0 : D + 1, off:512],
                                lhsT=vaug[:, tj, 2 * p + h, :],
                                rhs=pt[:, 512 * h + off : 512 * h + 512],
                                start=(tj == 0),
                                stop=(tj == ntj - 1),
                                skip_group_check=True,
                            )
                        for _ in range(2):
                            if dve_q:
                                dve_q.pop(0)()
                    # epilogue, ordered so the recip -> bc-broadcast ->
                    # norm-mul chain is always a full boundary ahead of its
                    # consumer and the PE never waits on it:
                    #   1. reciprocal of the denominator rows straight out of
                    #      PSUM (row 64 of each head's O^T accumulation)
                    #   2. compact both heads' O (shifted DVE copies, no DMA)
                    #   3. partition-broadcast of the reciprocals (stride-0
                    #      gpsimd DMA, feeds NEXT boundary's deferred norm)
                    #   4. deferred norm for the previous pair (GpSimd mul)
                    #   5. one outproj chunk-call from the FIFO (two
                    #      boundaries behind its ci, so its yn lhsT is ready)
                    yraw = yrawp.tile([128, 512], F, tag="yraw", name="yraw")
                    d_sb = dpool.tile([128, 2048], F, tag="D", name="D")
                    # denominator rows PSUM -> SBUF (plain shifted DVE copies;
                    # the custom-DVE reciprocal can't read PSUM), then ONE
                    # streaming-rate approx reciprocal over both heads' rows
                    dve_q.append(
                        lambda d=d_sb, o=o_ps[0]: nc.vector.tensor_copy(
                            d[0:1, 0:512], o[64:65, :]
                        )
                    )
                    dve_q.append(
                        lambda y=yraw, o=o_ps[0]: nc.vector.tensor_copy(
                            y[0:64, :], o[0:64, :]
                        )
                    )
                    dve_q.append(
                        lambda d=d_sb, o=o_ps[1]: nc.vector.tensor_copy(
                            d[0:1, 512:1024], o[64:65, :]
                        )
                    )
                    dve_q.append(
                        lambda y=yraw, o=o_ps[1]: nc.vector.tensor_copy(
                            y[64:128, :], o[0:64, :]
                        )
                    )
                    dve_q.append(
                        lambda d=d_sb: nc.vector.reciprocal_approx_fast(
                            out=d[0:1, 1024:2048], in_=d[0:1, 0:1024]
                        )
                    )
                    bc = bcpool.tile([128, 512], F, tag="bc", name="bc")

                    def mk_bc(d_sb, bc, h):
                        def f():
                            sl = d_sb[0:1, 1024 + 512 * h : 1024 + 512 * h + 512]
                            bsrc = bass.AP(
                                tensor=sl.tensor,
                                offset=sl.offset,
                                ap=[list(sl.ap[0]), [0, 64], [1, 512]],
                            )
                            nc.gpsimd.dma_start(
                                out=bc[64 * h : 64 * h + 64, :], in_=bsrc
                            )

                        return f

                    for h in range(2):
                        dve_q.append(mk_bc(d_sb, bc, h))

                    def mk_norm(pyn, pyraw, pbc):
                        def f():
                            pynorm = ynp.tile([128, 512], FR, tag="yn", name="yn")
                            nc.gpsimd.tensor_mul(pynorm, pyraw, pbc)
                            pyn.append(pynorm)

                        return f

                    if pending_norm is not None:
                        dve_q.append(mk_norm(*pending_norm))
                        pending_norm = None
                    if ci == 3:
                        dve_q.append(mk_norm(yn, yraw, bc))
                    else:
                        pending_norm = (yn, yraw, bc)
                    ncalls = 2 if len(outq) > 1 else 1
                    for _ in range(ncalls):
                        if not (
                            outq
                            and outq[0][1] <= ci - 1
                            and (p >= 1 or outq[0][1] <= ci - 2)
                        ):
                            break
                        pyn_l, pci, chunk = outq[0]
                        while len(pyn_l) < 4 and dve_q:
                            dve_q.pop(0)()
                        emit_outproj_chunk(pci, pyn_l, chunk)
                        if chunk == 3:
                            outq.pop(0)
                        else:
                            outq[0][2] = chunk + 1

                outq.append([yn, ci, 0])
            for pyn_l, pci, chunk in [
                (q[0], q[1], c) for q in outq for c in range(q[2], 4)
            ]:
                while len(pyn_l) < 4 and dve_q:
                    dve_q.pop(0)()
                emit_outproj_chunk(pci, pyn_l, chunk)
            while dve_q:
                dve_q.pop(0)()

    nc.compile()
    return nc


def _get_nc(with_bias: bool):
    if with_bias not in _NC_CACHE:
        _NC_CACHE[with_bias] = _build_nc(with_bias)
    return _NC_CACHE[with_bias]


def _rope_tables():
    half = D // 2
    i = np.arange(half, dtype=np.float32)
    expo = (2.0 * i / np.float32(D)).astype(np.float32)
    alpha = (1.0 / (np.float32(10000.0) ** expo)).astype(np.float32)
    ang = (np.arange(T, dtype=np.float32)[:, None] * alpha[None, :]).astype(np.float32)
    cosv = np.cos(ang).astype(np.float32).T  # [32, T]
    sinv = np.sin(ang).astype(np.float32).T
    c64 = np.concatenate([cosv, cosv], axis=0)  # [64, T]
    s64 = np.concatenate([-sinv, sinv], axis=0)
    ropeC = np.ascontiguousarray(np.concatenate([c64, c64], axis=0))  # [128, T]
    ropeS = np.ascontiguousarray(np.concatenate([s64, s64], axis=0))
    import ml_dtypes

    return ropeC.astype(ml_dtypes.bfloat16), ropeS.astype(ml_dtypes.bfloat16)


import ml_dtypes


def _round_fp32r(a):
    """Cast host data to the matmul operand dtype (bf16)."""
    return np.ascontiguousarray(
        np.asarray(a, dtype=np.float32).astype(ml_dtypes.bfloat16)
    )


def _swizzle_w(wT):
    """[KC*128, M] -> [128, KC*M] with fully-contiguous per-partition lines."""
    kc = wT.shape[0] // 128
    m = wT.shape[1]
    return np.ascontiguousarray(
        wT.reshape(kc, 128, m).transpose(1, 0, 2).reshape(128, kc * m)
    )


def _swizzle_x(xb):
    """[KC*128, T] -> [128, 2, KC*1024]: partition-major, T-half-major, then
    (chunk, within-half-col) contiguous."""
    kc = xb.shape[0] // 128
    return np.ascontiguousarray(
        xb.reshape(kc, 128, 2, 1024).transpose(1, 2, 0, 3).reshape(128, 2, kc * 1024)
    )


def _make_in_maps(x, wq, bq, wk, bk, wv, bv, wp, with_bias):
    ropeC, ropeS = _rope_tables()
    # 0/1 causal keep-mask for the diagonal 128x128 block: 1 where j <= i
    # (keep: key j, query i), 0 where masked
    mask01 = np.triu(np.ones((128, 128), np.float32)).astype(ml_dtypes.bfloat16)
    in_maps = []
    for b in range(B):
        xb = np.ascontiguousarray(x[b].T.astype(np.float32, copy=False))  # [C, T]
        if with_bias:
            aug = np.zeros((9 * 128 - C, T), np.float32)
            aug[0, :] = 1.0
            xb = np.concatenate([xb, aug], axis=0)
        xbs = _swizzle_x(_round_fp32r(xb))
        for g in range(2):
            sl = slice(g * CL, (g + 1) * CL)
            wqTc = np.ascontiguousarray(wq[sl, :].T.astype(np.float32, copy=False))
            wkTc = np.ascontiguousarray(wk[sl, :].T.astype(np.float32, copy=False))
            wvTc = np.ascontiguousarray(wv[sl, :].T.astype(np.float32, copy=False))
            if with_bias:
                npad = 9 * 128 - C

                def _aug_w(wT, bias):
                    a = np.zeros((npad, CL), np.float32)
                    a[0, :] = bias[sl].astype(np.float32, copy=False)
                    return np.ascontiguousarray(np.concatenate([wT, a], axis=0))

                wqTc = _aug_w(wqTc, bq)
                wkTc = _aug_w(wkTc, bk)
                wvTc = _aug_w(wvTc, bv)
            wpTc = np.ascontiguousarray(wp[:, sl].T.astype(np.float32, copy=False))
            in_maps.append(
                {
                    "xT": xbs,
                    "wqT": _swizzle_w(_round_fp32r(wqTc)),
                    "wkT": _swizzle_w(_round_fp32r(wkTc)),
                    "wvT": _swizzle_w(_round_fp32r(wvTc)),
                    "wpT": _swizzle_w(_round_fp32r(wpTc)),
                    "ropeC": ropeC,
                    "ropeS": ropeS,
                    "mask01": mask01,
                }
            )
    return in_maps


def _gather(results, bp):
    out = np.empty((B, T, C), dtype=np.float32)
    bp32 = np.asarray(bp, dtype=np.float32)
    for b in range(B):
        out[b] = results[2 * b]["out"] + results[2 * b + 1]["out"] + bp32
    return out


def run(x, wq, bq, wk, bk, wv, bv, wp, bp, trace=False, **kw):
    """Build/compile (cached), run on 8 cores, gather. Returns (out, results)."""
    arrs = [np.asarray(a) for a in (x, wq, bq, wk, bk, wv, bv, wp, bp)]
    x, wq, bq, wk, bk, wv, bv, wp, bp = arrs
    with_bias = bool(np.any(bq) or np.any(bk) or np.any(bv))
    nc = _get_nc(with_bias)
    in_maps = _make_in_maps(x, wq, bq, wk, bk, wv, bv, wp, with_bias)
    res = run_bass_kernel_spmd(nc, in_maps, list(range(NCORES)), trace=trace, **kw)
    return _gather(res.results, bp), res


def kernel(x, wq, bq, wk, bk, wv, bv, wp, bp):
    out, _ = run(x, wq, bq, wk, bk, wv, bv, wp, bp)
    return out


# revision 9
# speedup vs baseline: 1.2740x; 1.0198x over previous
"""Causal self-attention (B=4, T=2048, C=1024, H=16, rope) on 8 trn2 cores.

Sharding: data-parallel over B (4) x tensor-parallel over heads (2 groups of
8 heads). Core (b, g) computes its batch's Q/K/V for its 8 heads, the full
causal attention for those heads, and a partial output projection
(y_heads @ wp_cols.T). Host sums the two head-group partials per batch and
adds the output bias.

Device layout notes:
  - Q^T/K^T are kept as [c_out, t] tiles (partition = head-dim, 2 heads per
    128-partition tile) so QK^T needs no transposes; scores are computed as
    S^T[j, i] tiles (partition = key pos, free = query pos).
  - RoPE is applied as qt = ps*C + swap(ps)*S' where swap() is realized by
    32-row SBUF->SBUF DMAs on the sync queue and C/S' are host-precomputed
    tables.
  - Softmax denominators come free from an extra all-ones column appended to
    V (row 64 of the O^T accumulation); no max-subtraction is needed because
    the logits are bounded for this problem scale.
  - The causal mask for diagonal 128x128 blocks is applied AFTER the exp as
    a 0/1 tensor_mul on GpSimd (saves the PE mask-matmuls; exp of unmasked
    entries is bounded ~e^3 so no overflow).
  - The denominator chain is: DVE reciprocal straight off the O^T PSUM row,
    then a partition-broadcast DMA (stride-0 read) on the gpsimd queue, then
    a GpSimd norm-multiply -- no pack/unpack round trips.
  - HBM tensors are laid out so every initial DMA reads fully-contiguous
    per-partition lines (8-16 KiB descriptors, full HBM rate).
  - Matmul operands are bf16 (full-rate PE path); accumulation stays fp32 in
    PSUM. A short PE pre-warm covers the initial DMA window so the HAM clock
    gate is at 8/8 when real work arrives.
"""

import sys

if "/opt/trn_rl_repo" not in sys.path:
    sys.path.insert(0, "/opt/trn_rl_repo")

from contextlib import ExitStack

import numpy as np

import concourse.bass as bass
import concourse.mybir as mybir
from concourse import bacc
from concourse.bass_utils import run_bass_kernel_spmd
from concourse.tile import TileContext

B, T, C = 4, 2048, 1024
H = 16
D = 64
NCORES = 8
CL = C // 2  # per-core c_out (8 heads * 64)
HL = 8  # local heads
F = mybir.dt.float32
FR = mybir.dt.bfloat16  # matmul operand dtype

_NC_CACHE = {}


def _build_nc(with_bias: bool):
    KC = 9 if with_bias else 8  # c_in chunks of 128 (one extra for bias row)
    nc = bacc.Bacc("TRN2", debug=False, num_devices=NCORES)

    # Host-side pre-swizzled layouts: fully-contiguous per-partition lines so
    # each initial DMA moves 8-16KB per partition in one descriptor.
    xT = nc.declare_dram_parameter("xT", [128, 2, KC * 1024], FR, isOutput=False).ap()
    wqT = nc.declare_dram_parameter("wqT", [128, KC * CL], FR, isOutput=False).ap()
    wkT = nc.declare_dram_parameter("wkT", [128, KC * CL], FR, isOutput=False).ap()
    wvT = nc.declare_dram_parameter("wvT", [128, KC * CL], FR, isOutput=False).ap()
    wpT = nc.declare_dram_parameter("wpT", [128, 4 * C], FR, isOutput=False).ap()
    ropeC = nc.declare_dram_parameter("ropeC", [128, T], FR, isOutput=False).ap()
    ropeS = nc.declare_dram_parameter("ropeS", [128, T], FR, isOutput=False).ap()
    mask01 = nc.declare_dram_parameter("mask01", [128, 128], FR, isOutput=False).ap()
    out = nc.declare_dram_parameter("out", [T, C], F, isOutput=True).ap()

    EXP = mybir.ActivationFunctionType.Exp
    scale = 1.0 / float(np.sqrt(D))

    with TileContext(nc) as tc:
        with ExitStack() as ctx:
            # pools that live across both phases
            qk_pool = ctx.enter_context(tc.tile_pool(name="qk", bufs=1))
            v_pool = ctx.enter_context(tc.tile_pool(name="v", bufs=1))
            c2 = ctx.enter_context(tc.tile_pool(name="c2", bufs=1))

            qt_sb = [
                qk_pool.tile([128, T], FR, tag=f"qt{m}", name=f"qt{m}")
                for m in range(4)
            ]
            kt_sb = [
                qk_pool.tile([128, T], FR, tag=f"kt{m}", name=f"kt{m}")
                for m in range(4)
            ]
            # all 16 V tiles in one buffer: [t-block, head, D+1]; col D is the
            # all-ones denominator column
            vaug = v_pool.tile([128, 16, HL, D + 1], FR, tag="va", name="va")

            wp_sb = c2.tile([128, 4 * C], FR, tag="wp", name="wp")
            mk_sb = c2.tile([128, 128], FR, tag="mk", name="mk")

            # ---------------- phase 1: QKV projections + rope ----------------
            with ExitStack() as p1:
                wpool = p1.enter_context(tc.tile_pool(name="w", bufs=1))
                xpool = p1.enter_context(tc.tile_pool(name="x", bufs=1))
                rpool = p1.enter_context(tc.tile_pool(name="rope", bufs=1))
                tpool = p1.enter_context(tc.tile_pool(name="t1", bufs=3))
                ps1 = p1.enter_context(tc.tile_pool(name="ps1", bufs=4, space="PSUM"))
                wrm = p1.enter_context(tc.tile_pool(name="wrm", bufs=1, space="PSUM"))

                # PE pre-warm: keep the HAM clock gate at 8/8 while the input
                # DMAs land, so the first real matmuls run at 2.4 GHz.
                warm_sb = tpool.tile([128, 512], FR, tag="warm", name="warm")
                nc.vector.memset(warm_sb, 0.0)
                warm_ps = wrm.tile([128, 512], F, tag="wps", name="wps")
                for _ in range(18):
                    nc.tensor.matmul(
                        warm_ps,
                        lhsT=warm_sb[:, 0:128],
                        rhs=warm_sb,
                        start=True,
                        stop=True,
                        skip_group_check=True,
                    )

                # initial loads: x halves on the sync queue, weights on the
                # scalar queue -- all transfers are fully contiguous per
                # partition so they run at HBM line rate. The first K matmul
                # needs only wk + x half 0 (~3MB).
                x_sb = xpool.tile([128, 2, KC * 1024], FR, tag="x", name="x")
                wk_sb = wpool.tile([128, KC * CL], FR, tag="wk", name="wk")
                wq_sb = wpool.tile([128, KC * CL], FR, tag="wq", name="wq")
                wv_sb = wpool.tile([128, KC * CL], FR, tag="wv", name="wv")
                nc.scalar.dma_start(out=wk_sb, in_=wkT)
                nc.sync.dma_start(out=x_sb[:, 0, :], in_=xT[:, 0, :])
                nc.scalar.dma_start(out=wq_sb, in_=wqT)
                nc.scalar.dma_start(out=x_sb[:, 1, :], in_=xT[:, 1, :])
                nc.scalar.dma_start(out=wv_sb, in_=wvT)
                nc.scalar.dma_start(out=wp_sb, in_=wpT)

                rc_sb = rpool.tile([128, T], FR, tag="rc", name="rc")
                rs_sb = rpool.tile([128, T], FR, tag="rs", name="rs")
                nc.gpsimd.dma_start(out=rc_sb, in_=ropeC)
                nc.gpsimd.dma_start(out=rs_sb, in_=ropeS)
                nc.gpsimd.dma_start(out=mk_sb, in_=mask01)

                # ones column of vaug (denominator trick)
                nc.vector.memset(vaug[:, :, :, D : D + 1], 1.0)

                def x_rhs(k, lo, hi):
                    """x^T slice [128, hi-lo] for contraction chunk k, T cols
                    [lo, hi) (must stay within one T-half)."""
                    h = lo // 1024
                    o = lo - 1024 * h
                    return x_sb[:, h, 1024 * k + o : 1024 * k + o + (hi - lo)]

                def emit_kq(m, t):
                    # K and Q tiles ([c_out, t] layout) + rope, paired per
                    # (m, t) so each 32-row rope swap DMA moves both
                    # projections at once (2KB per-partition lines). Swap DMAs
                    # ride the sync queue, which is idle after the x load.
                    kq_ps = []
                    for wsb in (wk_sb, wq_sb):
                        ps = ps1.tile([128, 512], F, tag="ps", name="ps")
                        for k in range(KC):
                            nc.tensor.matmul(
                                ps,
                                lhsT=wsb[:, CL * k + 128 * m : CL * k + 128 * (m + 1)],
                                rhs=x_rhs(k, 512 * t, 512 * (t + 1)),
                                start=(k == 0),
                                stop=(k == KC - 1),
                            )
                        kq_ps.append(ps)
                    cp = tpool.tile([128, 1024], FR, tag="cp", name="cp")
                    nc.scalar.copy(cp[:, 0:512], kq_ps[0])
                    nc.scalar.copy(cp[:, 512:1024], kq_ps[1])
                    sw = tpool.tile([128, 1024], FR, tag="sw", name="sw")
                    for a, b in ((0, 32), (32, 0), (64, 96), (96, 64)):
                        nc.sync.dma_start(out=sw[a : a + 32, :], in_=cp[b : b + 32, :])
                    ts = slice(512 * t, 512 * (t + 1))
                    t1 = tpool.tile([128, 1024], FR, tag="t1", name="t1")
                    nc.vector.tensor_mul(t1[:, 0:512], kq_ps[0], rc_sb[:, ts])
                    nc.vector.tensor_mul(t1[:, 512:1024], kq_ps[1], rc_sb[:, ts])
                    t2 = tpool.tile([128, 1024], FR, tag="t2", name="t2")
                    nc.gpsimd.tensor_mul(t2[:, 0:512], sw[:, 0:512], rs_sb[:, ts])
                    nc.gpsimd.tensor_mul(t2[:, 512:1024], sw[:, 512:1024], rs_sb[:, ts])
                    nc.vector.tensor_add(kt_sb[m][:, ts], t1[:, 0:512], t2[:, 0:512])
                    nc.vector.tensor_add(
                        qt_sb[m][:, ts], t1[:, 512:1024], t2[:, 512:1024]
                    )

                def emit_v(jj):
                    # V tile (natural [t, c_out] layout) -> vaug, extraction on
                    # the scalar engine
                    ps = ps1.tile([128, 512], F, tag="ps", name="ps")
                    for k in range(KC):
                        nc.tensor.matmul(
                            ps,
                            lhsT=x_rhs(k, 128 * jj, 128 * (jj + 1)),
                            rhs=wv_sb[:, CL * k : CL * (k + 1)],
                            start=(k == 0),
                            stop=(k == KC - 1),
                        )
                    nc.scalar.copy(
                        out=vaug[:, jj, :, 0:D],
                        in_=ps.rearrange("p (h d) -> p h d", h=HL),
                    )

                # interleave: K/Q for the first T-half, then V tiles of that
                # half (so attention for early ci can start while the second
                # half projects), then the rest.
                for m in range(4):
                    for t in (0, 1):
                        emit_kq(m, t)
                for jj in range(8):
                    emit_v(jj)
                for m in range(4):
                    for t in (2, 3):
                        emit_kq(m, t)
                for jj in range(8, 16):
                    emit_v(jj)

            # ---------------- phase 2: attention + output projection ---------
            ppool = ctx.enter_context(tc.tile_pool(name="pt", bufs=3))
            yrawp = ctx.enter_context(tc.tile_pool(name="yraw", bufs=4))
            ynp = ctx.enter_context(tc.tile_pool(name="yn", bufs=9))
            osbp = ctx.enter_context(tc.tile_pool(name="osb", bufs=3))
            dpool = ctx.enter_context(tc.tile_pool(name="dd", bufs=2))
            bcpool = ctx.enter_context(tc.tile_pool(name="bc", bufs=3))
            spool = ctx.enter_context(tc.tile_pool(name="sps", bufs=2, space="PSUM"))
            opool = ctx.enter_context(tc.tile_pool(name="ops", bufs=2, space="PSUM"))

            def emit_outproj_chunk(ci, yn, chunk):
                for g in (2 * chunk, 2 * chunk + 1):
                    tt, cc = g % 4, g // 4
                    pr = opool.tile([128, 512], F, tag="o", name="pr")
                    for p in range(4):
                        nc.tensor.matmul(
                            pr,
                            lhsT=yn[p][:, 128 * tt : 128 * (tt + 1)],
                            rhs=wp_sb[:, 1024 * p + 512 * cc : 1024 * p + 512 * (cc + 1)],
                            start=(p == 0),
                            stop=(p == 3),
                        )
                    osb = osbp.tile([128, 512], F, tag="osb", name="osb")
                    nc.vector.tensor_copy(osb, pr)
                    nc.sync.dma_start(
                        out=out[
                            512 * ci + 128 * tt : 512 * ci + 128 * (tt + 1),
                            512 * cc : 512 * (cc + 1),
                        ],
                        in_=osb,
                    )

            outq = []  # (yn_list, ci, next_chunk) FIFO of outproj work
            dve_q = []  # deferred boundary ops, drained 2 per tj iter
            pending_norm = None
            for ci in range(4):
                yn = []
                for p in range(4):
                    # bound the deferral window to one boundary's worth so
                    # pool reuse (WAR) tracking stays consistent with the
                    # actual emission order
                    while len(dve_q) > 7:
                        dve_q.pop(0)()
                    o_ps = opool.tile([128, 1024], F, tag="o", name="o")
                    ntj = 4 * ci + 4
                    for tj in range(ntj):
                        kk = tj - 4 * ci
                        off = 128 * max(kk, 0)
                        s_ps = spool.tile([128, 1024], F, tag="s", name="s")
                        for h in range(2):
                            nc.tensor.matmul(
                                s_ps[:, 512 * h + off : 512 * h + 512],
                                lhsT=kt_sb[p][
                                    64 * h : 64 * h + 64,
                                    128 * tj : 128 * (tj + 1),
                                ],
                                rhs=qt_sb[p][
                                    64 * h : 64 * h + 64,
                                    512 * ci + off : 512 * (ci + 1),
                                ],
                                start=True,
                                stop=True,
                                tile_position=(64 * h, 0),
                                skip_group_check=True,
                            )
                        pt = ppool.tile([128, 1024], FR, tag="pt", name="pt")
                        if kk < 0:
                            nc.scalar.activation(pt, s_ps, EXP, scale=scale)
                        else:
                            s_v = s_ps.rearrange("q (h n) -> q h n", h=2)[:, :, off:]
                            p_v = pt.rearrange("q (h n) -> q h n", h=2)[:, :, off:]
                            nc.scalar.activation(p_v, s_v, EXP, scale=scale)
                            # causal mask for the diagonal 128x128 block:
                            # multiply by the 0/1 upper-tri mask (split across
                            # DVE and GpSimd; keeps the PE out of the mask
                            # business)
                            for h, eng in ((0, nc.vector), (1, nc.gpsimd)):
                                eng.tensor_mul(
                                    pt[:, 512 * h + off : 512 * h + off + 128],
                                    pt[:, 512 * h + off : 512 * h + off + 128],
                                    mk_sb,
                                )
                        for h in range(2):
                            nc.tensor.matmul(
                                o_ps[0 : D + 1, 512 * h + off : 512 * h + 512],
                                lhsT=vaug[:, tj, 2 * p + h, :],
                                rhs=pt[:, 512 * h + off : 512 * h + 512],
                                start=(tj == 0),
                                stop=(tj == ntj - 1),
                                skip_group_check=True,
                            )
                        for _ in range(2):
                            if dve_q:
                                dve_q.pop(0)()
                    # epilogue, ordered so the recip -> bc-broadcast ->
                    # norm-mul chain is always a full boundary ahead of its
                    # consumer and the PE never waits on it:
                    #   1. reciprocal of the denominator rows straight out of
                    #      PSUM (row 64 of each head's O^T accumulation)
                    #   2. compact both heads' O (shifted DVE copies, no DMA)
                    #   3. partition-broadcast of the reciprocals (stride-0
                    #      gpsimd DMA, feeds NEXT boundary's deferred norm)
                    #   4. deferred norm for the previous pair (GpSimd mul)
                    #   5. one outproj chunk-call from the FIFO (two
                    #      boundaries behind its ci, so its yn lhsT is ready)
                    yraw = yrawp.tile([128, 512], F, tag="yraw", name="yraw")
                    d_sb = dpool.tile([128, 2048], F, tag="D", name="D")
                    # denominator rows PSUM -> SBUF (plain shifted DVE copies;
                    # the custom-DVE reciprocal can't read PSUM), then ONE
                    # streaming-rate approx reciprocal over both heads' rows
                    dve_q.append(
                        lambda d=d_sb, o=o_ps: nc.vector.tensor_copy(
                            d[0:1, 0:1024], o[64:65, :]
                        )
                    )
                    dve_q.append(
                        lambda y=yraw, o=o_ps: nc.vector.tensor_copy(
                            y[0:64, :], o[0:64, 0:512]
                        )
                    )
                    dve_q.append(
                        lambda y=yraw, o=o_ps: nc.vector.tensor_copy(
                            y[64:128, :], o[0:64, 512:1024]
                        )
                    )
                    dve_q.append(
                        lambda d=d_sb: nc.vector.reciprocal_approx_fast(
                            out=d[0:1, 1024:2048], in_=d[0:1, 0:1024]
                        )
                    )
                    bc = bcpool.tile([128, 512], F, tag="bc", name="bc")

                    def mk_bc(d_sb, bc, h):
                        def f():
                            sl = d_sb[0:1, 1024 + 512 * h : 1024 + 512 * h + 512]
                            bsrc = bass.AP(
                                tensor=sl.tensor,
                                offset=sl.offset,
                                ap=[list(sl.ap[0]), [0, 64], [1, 512]],
                            )
                            nc.sync.dma_start(
                                out=bc[64 * h : 64 * h + 64, :], in_=bsrc
                            )

                        return f

                    for h in range(2):
                        dve_q.append(mk_bc(d_sb, bc, h))

                    def mk_norm(pyn, pyraw, pbc):
                        def f():
                            pynorm = ynp.tile([128, 512], FR, tag="yn", name="yn")
                            nc.gpsimd.tensor_mul(pynorm, pyraw, pbc)
                            pyn.append(pynorm)

                        return f

                    if pending_norm is not None:
                        dve_q.append(mk_norm(*pending_norm))
                        pending_norm = None
                    if ci == 3:
                        dve_q.append(mk_norm(yn, yraw, bc))
                    else:
                        pending_norm = (yn, yraw, bc)
                    ncalls = 2 if len(outq) > 1 else 1
                    for _ in range(ncalls):
                        if not (
                            outq
                            and outq[0][1] <= ci - 1
                            and (p >= 1 or outq[0][1] <= ci - 2)
                        ):
                            break
                        pyn_l, pci, chunk = outq[0]
                        while len(pyn_l) < 4 and dve_q:
                            dve_q.pop(0)()
                        emit_outproj_chunk(pci, pyn_l, chunk)
                        if chunk == 3:
                            outq.pop(0)
                        else:
                            outq[0][2] = chunk + 1

                outq.append([yn, ci, 0])
            for pyn_l, pci, chunk in [
                (q[0], q[1], c) for q in outq for c in range(q[2], 4)
            ]:
                while len(pyn_l) < 4 and dve_q:
                    dve_q.pop(0)()
                emit_outproj_chunk(pci, pyn_l, chunk)
            while dve_q:
                dve_q.pop(0)()

    nc.compile()
    return nc


def _get_nc(with_bias: bool):
    if with_bias not in _NC_CACHE:
        _NC_CACHE[with_bias] = _build_nc(with_bias)
    return _NC_CACHE[with_bias]


def _rope_tables():
    half = D // 2
    i = np.arange(half, dtype=np.float32)
    expo = (2.0 * i / np.float32(D)).astype(np.float32)
    alpha = (1.0 / (np.float32(10000.0) ** expo)).astype(np.float32)
    ang = (np.arange(T, dtype=np.float32)[:, None] * alpha[None, :]).astype(np.float32)
    cosv = np.cos(ang).astype(np.float32).T  # [32, T]
    sinv = np.sin(ang).astype(np.float32).T
    c64 = np.concatenate([cosv, cosv], axis=0)  # [64, T]
    s64 = np.concatenate([-sinv, sinv], axis=0)
    ropeC = np.ascontiguousarray(np.concatenate([c64, c64], axis=0))  # [128, T]
    ropeS = np.ascontiguousarray(np.concatenate([s64, s64], axis=0))
    import ml_dtypes

    return ropeC.astype(ml_dtypes.bfloat16), ropeS.astype(ml_dtypes.bfloat16)


import ml_dtypes


def _round_fp32r(a):
    """Cast host data to the matmul operand dtype (bf16)."""
    return np.ascontiguousarray(
        np.asarray(a, dtype=np.float32).astype(ml_dtypes.bfloat16)
    )


def _swizzle_w(wT):
    """[KC*128, M] -> [128, KC*M] with fully-contiguous per-partition lines."""
    kc = wT.shape[0] // 128
    m = wT.shape[1]
    return np.ascontiguousarray(
        wT.reshape(kc, 128, m).transpose(1, 0, 2).reshape(128, kc * m)
    )


def _swizzle_x(xb):
    """[KC*128, T] -> [128, 2, KC*1024]: partition-major, T-half-major, then
    (chunk, within-half-col) contiguous."""
    kc = xb.shape[0] // 128
    return np.ascontiguousarray(
        xb.reshape(kc, 128, 2, 1024).transpose(1, 2, 0, 3).reshape(128, 2, kc * 1024)
    )


def _make_in_maps(x, wq, bq, wk, bk, wv, bv, wp, with_bias):
    ropeC, ropeS = _rope_tables()
    # 0/1 causal keep-mask for the diagonal 128x128 block: 1 where j <= i
    # (keep: key j, query i), 0 where masked
    mask01 = np.triu(np.ones((128, 128), np.float32)).astype(ml_dtypes.bfloat16)
    in_maps = []
    for b in range(B):
        xb = np.ascontiguousarray(x[b].T.astype(np.float32, copy=False))  # [C, T]
        if with_bias:
            aug = np.zeros((9 * 128 - C, T), np.float32)
            aug[0, :] = 1.0
            xb = np.concatenate([xb, aug], axis=0)
        xbs = _swizzle_x(_round_fp32r(xb))
        for g in range(2):
            sl = slice(g * CL, (g + 1) * CL)
            wqTc = np.ascontiguousarray(wq[sl, :].T.astype(np.float32, copy=False))
            wkTc = np.ascontiguousarray(wk[sl, :].T.astype(np.float32, copy=False))
            wvTc = np.ascontiguousarray(wv[sl, :].T.astype(np.float32, copy=False))
            if with_bias:
                npad = 9 * 128 - C

                def _aug_w(wT, bias):
                    a = np.zeros((npad, CL), np.float32)
                    a[0, :] = bias[sl].astype(np.float32, copy=False)
                    return np.ascontiguousarray(np.concatenate([wT, a], axis=0))

                wqTc = _aug_w(wqTc, bq)
                wkTc = _aug_w(wkTc, bk)
                wvTc = _aug_w(wvTc, bv)
            wpTc = np.ascontiguousarray(wp[:, sl].T.astype(np.float32, copy=False))
            in_maps.append(
                {
                    "xT": xbs,
                    "wqT": _swizzle_w(_round_fp32r(wqTc)),
                    "wkT": _swizzle_w(_round_fp32r(wkTc)),
                    "wvT": _swizzle_w(_round_fp32r(wvTc)),
                    "wpT": _swizzle_w(_round_fp32r(wpTc)),
                    "ropeC": ropeC,
                    "ropeS": ropeS,
                    "mask01": mask01,
                }
            )
    return in_maps


def _gather(results, bp):
    out = np.empty((B, T, C), dtype=np.float32)
    bp32 = np.asarray(bp, dtype=np.float32)
    for b in range(B):
        out[b] = results[2 * b]["out"] + results[2 * b + 1]["out"] + bp32
    return out


def run(x, wq, bq, wk, bk, wv, bv, wp, bp, trace=False, **kw):
    """Build/compile (cached), run on 8 cores, gather. Returns (out, results)."""
    arrs = [np.asarray(a) for a in (x, wq, bq, wk, bk, wv, bv, wp, bp)]
    x, wq, bq, wk, bk, wv, bv, wp, bp = arrs
    with_bias = bool(np.any(bq) or np.any(bk) or np.any(bv))
    nc = _get_nc(with_bias)
    in_maps = _make_in_maps(x, wq, bq, wk, bk, wv, bv, wp, with_bias)
    res = run_bass_kernel_spmd(nc, in_maps, list(range(NCORES)), trace=trace, **kw)
    return _gather(res.results, bp), res


def kernel(x, wq, bq, wk, bk, wv, bv, wp, bp):
    out, _ = run(x, wq, bq, wk, bk, wv, bv, wp, bp)
    return out


# revision 10
# speedup vs baseline: 1.4263x; 1.1196x over previous
"""Causal self-attention (B=4, T=2048, C=1024, H=16, rope) on 8 trn2 cores.

Sharding: data-parallel over B (4) x tensor-parallel over heads (2 groups of
8 heads). Core (b, g) computes its batch's Q/K/V for its 8 heads, the full
causal attention for those heads, and a partial output projection
(y_heads @ wp_cols.T). Host sums the two head-group partials per batch and
adds the output bias.

Device layout notes:
  - Q^T/K^T are kept as [c_out, t] tiles (partition = head-dim, 2 heads per
    128-partition tile) so QK^T needs no transposes; scores are computed as
    S^T[j, i] tiles (partition = key pos, free = query pos).
  - RoPE is applied as qt = ps*C + swap(ps)*S' where swap() is realized by
    32-row SBUF->SBUF DMAs on the sync queue and C/S' are host-precomputed
    tables.
  - Softmax denominators come free from an extra all-ones column appended to
    V (row 64 of the O^T accumulation); no max-subtraction is needed because
    the logits are bounded for this problem scale.
  - The causal mask for diagonal 128x128 blocks is applied AFTER the exp as
    a 0/1 tensor_mul on GpSimd (saves the PE mask-matmuls; exp of unmasked
    entries is bounded ~e^3 so no overflow).
  - The denominator chain is: DVE reciprocal straight off the O^T PSUM row,
    then a partition-broadcast DMA (stride-0 read) on the gpsimd queue, then
    a GpSimd norm-multiply -- no pack/unpack round trips.
  - HBM tensors are laid out so every initial DMA reads fully-contiguous
    per-partition lines (8-16 KiB descriptors, full HBM rate).
  - Matmul operands are bf16 (full-rate PE path); accumulation stays fp32 in
    PSUM. A short PE pre-warm covers the initial DMA window so the HAM clock
    gate is at 8/8 when real work arrives.
"""

import sys

if "/opt/trn_rl_repo" not in sys.path:
    sys.path.insert(0, "/opt/trn_rl_repo")

from contextlib import ExitStack

import numpy as np

import concourse.bass as bass
import concourse.mybir as mybir
from concourse import bacc
from concourse.bass_utils import run_bass_kernel_spmd
from concourse.tile import TileContext

B, T, C = 4, 2048, 1024
H = 16
D = 64
NCORES = 8
CL = C // 2  # per-core c_out (8 heads * 64)
HL = 8  # local heads
F = mybir.dt.float32
FR = mybir.dt.bfloat16  # matmul operand dtype

_NC_CACHE = {}


def _build_nc(with_bias: bool):
    KC = 9 if with_bias else 8  # c_in chunks of 128 (one extra for bias row)
    nc = bacc.Bacc("TRN2", debug=False, num_devices=NCORES)

    # Host-side pre-swizzled layouts: fully-contiguous per-partition lines so
    # each initial DMA moves 8-16KB per partition in one descriptor.
    xT = nc.declare_dram_parameter("xT", [128, 2, KC * 1024], FR, isOutput=False).ap()
    wqT = nc.declare_dram_parameter("wqT", [128, KC * CL], FR, isOutput=False).ap()
    wkT = nc.declare_dram_parameter("wkT", [128, KC * CL], FR, isOutput=False).ap()
    wvT = nc.declare_dram_parameter("wvT", [128, KC * CL], FR, isOutput=False).ap()
    wpT = nc.declare_dram_parameter("wpT", [128, 4 * C], FR, isOutput=False).ap()
    ropeC = nc.declare_dram_parameter("ropeC", [128, T], FR, isOutput=False).ap()
    ropeS = nc.declare_dram_parameter("ropeS", [128, T], FR, isOutput=False).ap()
    mask01 = nc.declare_dram_parameter("mask01", [128, 128], FR, isOutput=False).ap()
    out = nc.declare_dram_parameter("out", [T, C], F, isOutput=True).ap()

    EXP = mybir.ActivationFunctionType.Exp
    scale = 1.0 / float(np.sqrt(D))

    with TileContext(nc) as tc:
        with ExitStack() as ctx:
            # pools that live across both phases
            qk_pool = ctx.enter_context(tc.tile_pool(name="qk", bufs=1))
            v_pool = ctx.enter_context(tc.tile_pool(name="v", bufs=1))
            c2 = ctx.enter_context(tc.tile_pool(name="c2", bufs=1))

            qt_sb = [
                qk_pool.tile([128, T], FR, tag=f"qt{m}", name=f"qt{m}")
                for m in range(4)
            ]
            kt_sb = [
                qk_pool.tile([128, T], FR, tag=f"kt{m}", name=f"kt{m}")
                for m in range(4)
            ]
            # all 16 V tiles in one buffer: [t-block, head, D+1]; col D is the
            # all-ones denominator column
            vaug = v_pool.tile([128, 16, HL, D + 1], FR, tag="va", name="va")

            wp_sb = c2.tile([128, 4 * C], FR, tag="wp", name="wp")
            mk_sb = c2.tile([128, 128], FR, tag="mk", name="mk")

            # ---------------- phase 1: QKV projections + rope ----------------
            with ExitStack() as p1:
                wpool = p1.enter_context(tc.tile_pool(name="w", bufs=1))
                xpool = p1.enter_context(tc.tile_pool(name="x", bufs=1))
                rpool = p1.enter_context(tc.tile_pool(name="rope", bufs=1))
                tpool = p1.enter_context(tc.tile_pool(name="t1", bufs=3))
                ps1 = p1.enter_context(tc.tile_pool(name="ps1", bufs=4, space="PSUM"))
                wrm = p1.enter_context(tc.tile_pool(name="wrm", bufs=1, space="PSUM"))

                # PE pre-warm: keep the HAM clock gate at 8/8 while the input
                # DMAs land, so the first real matmuls run at 2.4 GHz.
                warm_sb = tpool.tile([128, 512], FR, tag="warm", name="warm")
                nc.vector.memset(warm_sb, 0.0)
                warm_ps = wrm.tile([128, 512], F, tag="wps", name="wps")
                for _ in range(18):
                    nc.tensor.matmul(
                        warm_ps,
                        lhsT=warm_sb[:, 0:128],
                        rhs=warm_sb,
                        start=True,
                        stop=True,
                        skip_group_check=True,
                    )

                # initial loads: x halves on the sync queue, weights on the
                # scalar queue -- all transfers are fully contiguous per
                # partition so they run at HBM line rate. The first K matmul
                # needs only wk + x half 0 (~3MB).
                x_sb = xpool.tile([128, 2, KC * 1024], FR, tag="x", name="x")
                wk_sb = wpool.tile([128, KC * CL], FR, tag="wk", name="wk")
                wq_sb = wpool.tile([128, KC * CL], FR, tag="wq", name="wq")
                wv_sb = wpool.tile([128, KC * CL], FR, tag="wv", name="wv")
                nc.scalar.dma_start(out=wk_sb, in_=wkT)
                nc.sync.dma_start(out=x_sb[:, 0, :], in_=xT[:, 0, :])
                nc.scalar.dma_start(out=wq_sb, in_=wqT)
                nc.scalar.dma_start(out=x_sb[:, 1, :], in_=xT[:, 1, :])
                nc.scalar.dma_start(out=wv_sb, in_=wvT)
                nc.scalar.dma_start(out=wp_sb, in_=wpT)

                rc_sb = rpool.tile([128, T], FR, tag="rc", name="rc")
                rs_sb = rpool.tile([128, T], FR, tag="rs", name="rs")
                nc.gpsimd.dma_start(out=rc_sb, in_=ropeC)
                nc.gpsimd.dma_start(out=rs_sb, in_=ropeS)
                nc.gpsimd.dma_start(out=mk_sb, in_=mask01)

                # ones column of vaug (denominator trick)
                nc.vector.memset(vaug[:, :, :, D : D + 1], 1.0)

                def x_rhs(k, lo, hi):
                    """x^T slice [128, hi-lo] for contraction chunk k, T cols
                    [lo, hi) (must stay within one T-half)."""
                    h = lo // 1024
                    o = lo - 1024 * h
                    return x_sb[:, h, 1024 * k + o : 1024 * k + o + (hi - lo)]

                def emit_kq(m, t):
                    # K and Q tiles ([c_out, t] layout) + rope, paired per
                    # (m, t) so each 32-row rope swap DMA moves both
                    # projections at once (2KB per-partition lines). Swap DMAs
                    # ride the sync queue, which is idle after the x load.
                    kq_ps = []
                    for wsb in (wk_sb, wq_sb):
                        ps = ps1.tile([128, 512], F, tag="ps", name="ps")
                        for k in range(KC):
                            nc.tensor.matmul(
                                ps,
                                lhsT=wsb[:, CL * k + 128 * m : CL * k + 128 * (m + 1)],
                                rhs=x_rhs(k, 512 * t, 512 * (t + 1)),
                                start=(k == 0),
                                stop=(k == KC - 1),
                            )
                        kq_ps.append(ps)
                    cp = tpool.tile([128, 1024], FR, tag="cp", name="cp")
                    nc.scalar.copy(cp[:, 0:512], kq_ps[0])
                    nc.scalar.copy(cp[:, 512:1024], kq_ps[1])
                    sw = tpool.tile([128, 1024], FR, tag="sw", name="sw")
                    for a, b in ((0, 32), (32, 0), (64, 96), (96, 64)):
                        nc.sync.dma_start(out=sw[a : a + 32, :], in_=cp[b : b + 32, :])
                    ts = slice(512 * t, 512 * (t + 1))
                    t1 = tpool.tile([128, 1024], FR, tag="t1", name="t1")
                    nc.vector.tensor_mul(t1[:, 0:512], kq_ps[0], rc_sb[:, ts])
                    nc.vector.tensor_mul(t1[:, 512:1024], kq_ps[1], rc_sb[:, ts])
                    t2 = tpool.tile([128, 1024], FR, tag="t2", name="t2")
                    nc.gpsimd.tensor_mul(t2[:, 0:512], sw[:, 0:512], rs_sb[:, ts])
                    nc.gpsimd.tensor_mul(t2[:, 512:1024], sw[:, 512:1024], rs_sb[:, ts])
                    nc.vector.tensor_add(kt_sb[m][:, ts], t1[:, 0:512], t2[:, 0:512])
                    nc.vector.tensor_add(
                        qt_sb[m][:, ts], t1[:, 512:1024], t2[:, 512:1024]
                    )

                def emit_v(jj):
                    # V tile (natural [t, c_out] layout) -> vaug, extraction on
                    # the scalar engine
                    ps = ps1.tile([128, 512], F, tag="ps", name="ps")
                    for k in range(KC):
                        nc.tensor.matmul(
                            ps,
                            lhsT=x_rhs(k, 128 * jj, 128 * (jj + 1)),
                            rhs=wv_sb[:, CL * k : CL * (k + 1)],
                            start=(k == 0),
                            stop=(k == KC - 1),
                        )
                    nc.scalar.copy(
                        out=vaug[:, jj, :, 0:D],
                        in_=ps.rearrange("p (h d) -> p h d", h=HL),
                    )

                # interleave: K/Q for the first T-half, then V tiles of that
                # half (so attention for early ci can start while the second
                # half projects), then the rest.
                for m in range(4):
                    for t in (0, 1):
                        emit_kq(m, t)
                for jj in range(8):
                    emit_v(jj)
                for m in range(4):
                    for t in (2, 3):
                        emit_kq(m, t)
                for jj in range(8, 16):
                    emit_v(jj)

            # ---------------- phase 2: attention + output projection ---------
            ppool = ctx.enter_context(tc.tile_pool(name="pt", bufs=3))
            yrawp = ctx.enter_context(tc.tile_pool(name="yraw", bufs=4))
            ynp = ctx.enter_context(tc.tile_pool(name="yn", bufs=9))
            osbp = ctx.enter_context(tc.tile_pool(name="osb", bufs=3))
            dpool = ctx.enter_context(tc.tile_pool(name="dd", bufs=2))
            bcpool = ctx.enter_context(tc.tile_pool(name="bc", bufs=3))
            spool = ctx.enter_context(tc.tile_pool(name="sps", bufs=2, space="PSUM"))
            opool = ctx.enter_context(tc.tile_pool(name="ops", bufs=2, space="PSUM"))

            def emit_outproj_chunk(ci, yn, chunk):
                for g in (2 * chunk, 2 * chunk + 1):
                    tt, cc = g % 4, g // 4
                    pr = opool.tile([128, 512], F, tag="o", name="pr")
                    for p in range(4):
                        nc.tensor.matmul(
                            pr,
                            lhsT=yn[p][:, 128 * tt : 128 * (tt + 1)],
                            rhs=wp_sb[:, 1024 * p + 512 * cc : 1024 * p + 512 * (cc + 1)],
                            start=(p == 0),
                            stop=(p == 3),
                        )
                    osb = osbp.tile([128, 512], F, tag="osb", name="osb")
                    nc.vector.tensor_copy(osb, pr)
                    nc.sync.dma_start(
                        out=out[
                            512 * ci + 128 * tt : 512 * ci + 128 * (tt + 1),
                            512 * cc : 512 * (cc + 1),
                        ],
                        in_=osb,
                    )

            outq = []  # (yn_list, ci, next_chunk) FIFO of outproj work
            dve_q = []  # deferred boundary ops, drained 2 per tj iter
            pending_norm = None
            for ci in range(4):
                yn = []
                for p in range(4):
                    # bound the deferral window to one boundary's worth so
                    # pool reuse (WAR) tracking stays consistent with the
                    # actual emission order
                    while len(dve_q) > 7:
                        dve_q.pop(0)()
                    o_ps = opool.tile([128, 1024], F, tag="o", name="o")
                    ntj = 4 * ci + 4
                    for tj in range(ntj):
                        kk = tj - 4 * ci
                        off = 128 * max(kk, 0)
                        s_ps = spool.tile([128, 1024], F, tag="s", name="s")
                        for h in range(2):
                            nc.tensor.matmul(
                                s_ps[:, 512 * h + off : 512 * h + 512],
                                lhsT=kt_sb[p][
                                    64 * h : 64 * h + 64,
                                    128 * tj : 128 * (tj + 1),
                                ],
                                rhs=qt_sb[p][
                                    64 * h : 64 * h + 64,
                                    512 * ci + off : 512 * (ci + 1),
                                ],
                                start=True,
                                stop=True,
                                tile_position=(64 * h, 0),
                                skip_group_check=True,
                            )
                        pt = ppool.tile([128, 1024], FR, tag="pt", name="pt")
                        if kk < 0:
                            nc.scalar.activation(pt, s_ps, EXP, scale=scale)
                        else:
                            s_v = s_ps.rearrange("q (h n) -> q h n", h=2)[:, :, off:]
                            p_v = pt.rearrange("q (h n) -> q h n", h=2)[:, :, off:]
                            nc.scalar.activation(p_v, s_v, EXP, scale=scale)
                            # causal mask for the diagonal 128x128 block:
                            # multiply by the 0/1 upper-tri mask (split across
                            # DVE and GpSimd; keeps the PE out of the mask
                            # business)
                            for h, eng in ((0, nc.vector), (1, nc.vector)):
                                eng.tensor_mul(
                                    pt[:, 512 * h + off : 512 * h + off + 128],
                                    pt[:, 512 * h + off : 512 * h + off + 128],
                                    mk_sb,
                                )
                        for h in range(2):
                            nc.tensor.matmul(
                                o_ps[0 : D + 1, 512 * h + off : 512 * h + 512],
                                lhsT=vaug[:, tj, 2 * p + h, :],
                                rhs=pt[:, 512 * h + off : 512 * h + 512],
                                start=(tj == 0),
                                stop=(tj == ntj - 1),
                                skip_group_check=True,
                            )
                        for _ in range(3 if ci == 3 else 2):
                            if dve_q:
                                dve_q.pop(0)()
                    # epilogue, ordered so the recip -> bc-broadcast ->
                    # norm-mul chain is always a full boundary ahead of its
                    # consumer and the PE never waits on it:
                    #   1. reciprocal of the denominator rows straight out of
                    #      PSUM (row 64 of each head's O^T accumulation)
                    #   2. compact both heads' O (shifted DVE copies, no DMA)
                    #   3. partition-broadcast of the reciprocals (stride-0
                    #      gpsimd DMA, feeds NEXT boundary's deferred norm)
                    #   4. deferred norm for the previous pair (GpSimd mul)
                    #   5. one outproj chunk-call from the FIFO (two
                    #      boundaries behind its ci, so its yn lhsT is ready)
                    yraw = yrawp.tile([128, 512], F, tag="yraw", name="yraw")
                    d_sb = dpool.tile([128, 2048], F, tag="D", name="D")
                    # denominator rows PSUM -> SBUF (plain shifted DVE copies;
                    # the custom-DVE reciprocal can't read PSUM), then ONE
                    # streaming-rate approx reciprocal over both heads' rows
                    dve_q.append(
                        lambda d=d_sb, o=o_ps: nc.vector.tensor_copy(
                            d[0:1, 0:1024], o[64:65, :]
                        )
                    )
                    dve_q.append(
                        lambda y=yraw, o=o_ps: nc.vector.tensor_copy(
                            y[0:64, :], o[0:64, 0:512]
                        )
                    )
                    dve_q.append(
                        lambda y=yraw, o=o_ps: nc.vector.tensor_copy(
                            y[64:128, :], o[0:64, 512:1024]
                        )
                    )
                    dve_q.append(
                        lambda d=d_sb: nc.vector.reciprocal_approx_fast(
                            out=d[0:1, 1024:2048], in_=d[0:1, 0:1024]
                        )
                    )
                    bc = bcpool.tile([128, 512], F, tag="bc", name="bc")

                    def mk_bc(d_sb, bc, h):
                        def f():
                            sl = d_sb[0:1, 1024 + 512 * h : 1024 + 512 * h + 512]
                            bsrc = bass.AP(
                                tensor=sl.tensor,
                                offset=sl.offset,
                                ap=[list(sl.ap[0]), [0, 64], [1, 512]],
                            )
                            nc.gpsimd.dma_start(
                                out=bc[64 * h : 64 * h + 64, :], in_=bsrc
                            )

                        return f

                    for h in range(2):
                        dve_q.append(mk_bc(d_sb, bc, h))

                    def mk_norm(pyn, pyraw, pbc):
                        def f():
                            pynorm = ynp.tile([128, 512], FR, tag="yn", name="yn")
                            nc.gpsimd.tensor_mul(pynorm, pyraw, pbc)
                            pyn.append(pynorm)

                        return f

                    if pending_norm is not None:
                        dve_q.append(mk_norm(*pending_norm))
                        pending_norm = None
                    if ci == 3:
                        dve_q.append(mk_norm(yn, yraw, bc))
                    else:
                        pending_norm = (yn, yraw, bc)
                    ncalls = 2 if len(outq) > 1 else 1
                    for _ in range(ncalls):
                        if not (
                            outq
                            and outq[0][1] <= ci - 1
                            and (p >= 1 or outq[0][1] <= ci - 2)
                        ):
                            break
                        pyn_l, pci, chunk = outq[0]
                        while len(pyn_l) < 4 and dve_q:
                            dve_q.pop(0)()
                        emit_outproj_chunk(pci, pyn_l, chunk)
                        if chunk == 3:
                            outq.pop(0)
                        else:
                            outq[0][2] = chunk + 1

                outq.append([yn, ci, 0])
            for pyn_l, pci, chunk in [
                (q[0], q[1], c) for q in outq for c in range(q[2], 4)
            ]:
                while len(pyn_l) < 4 and dve_q:
                    dve_q.pop(0)()
                emit_outproj_chunk(pci, pyn_l, chunk)
            while dve_q:
                dve_q.pop(0)()

    nc.compile()
    return nc


def _get_nc(with_bias: bool):
    if with_bias not in _NC_CACHE:
        _NC_CACHE[with_bias] = _build_nc(with_bias)
    return _NC_CACHE[with_bias]


def _rope_tables():
    half = D // 2
    i = np.arange(half, dtype=np.float32)
    expo = (2.0 * i / np.float32(D)).astype(np.float32)
    alpha = (1.0 / (np.float32(10000.0) ** expo)).astype(np.float32)
    ang = (np.arange(T, dtype=np.float32)[:, None] * alpha[None, :]).astype(np.float32)
    cosv = np.cos(ang).astype(np.float32).T  # [32, T]
    sinv = np.sin(ang).astype(np.float32).T
    c64 = np.concatenate([cosv, cosv], axis=0)  # [64, T]
    s64 = np.concatenate([-sinv, sinv], axis=0)
    ropeC = np.ascontiguousarray(np.concatenate([c64, c64], axis=0))  # [128, T]
    ropeS = np.ascontiguousarray(np.concatenate([s64, s64], axis=0))
    import ml_dtypes

    return ropeC.astype(ml_dtypes.bfloat16), ropeS.astype(ml_dtypes.bfloat16)


import ml_dtypes


def _round_fp32r(a):
    """Cast host data to the matmul operand dtype (bf16)."""
    return np.ascontiguousarray(
        np.asarray(a, dtype=np.float32).astype(ml_dtypes.bfloat16)
    )


def _swizzle_w(wT):
    """[KC*128, M] -> [128, KC*M] with fully-contiguous per-partition lines."""
    kc = wT.shape[0] // 128
    m = wT.shape[1]
    return np.ascontiguousarray(
        wT.reshape(kc, 128, m).transpose(1, 0, 2).reshape(128, kc * m)
    )


def _swizzle_x(xb):
    """[KC*128, T] -> [128, 2, KC*1024]: partition-major, T-half-major, then
    (chunk, within-half-col) contiguous."""
    kc = xb.shape[0] // 128
    return np.ascontiguousarray(
        xb.reshape(kc, 128, 2, 1024).transpose(1, 2, 0, 3).reshape(128, 2, kc * 1024)
    )


def _make_in_maps(x, wq, bq, wk, bk, wv, bv, wp, with_bias):
    ropeC, ropeS = _rope_tables()
    # 0/1 causal keep-mask for the diagonal 128x128 block: 1 where j <= i
    # (keep: key j, query i), 0 where masked
    mask01 = np.triu(np.ones((128, 128), np.float32)).astype(ml_dtypes.bfloat16)
    in_maps = []
    for b in range(B):
        xb = np.ascontiguousarray(x[b].T.astype(np.float32, copy=False))  # [C, T]
        if with_bias:
            aug = np.zeros((9 * 128 - C, T), np.float32)
            aug[0, :] = 1.0
            xb = np.concatenate([xb, aug], axis=0)
        xbs = _swizzle_x(_round_fp32r(xb))
        for g in range(2):
            sl = slice(g * CL, (g + 1) * CL)
            wqTc = np.ascontiguousarray(wq[sl, :].T.astype(np.float32, copy=False))
            wkTc = np.ascontiguousarray(wk[sl, :].T.astype(np.float32, copy=False))
            wvTc = np.ascontiguousarray(wv[sl, :].T.astype(np.float32, copy=False))
            if with_bias:
                npad = 9 * 128 - C

                def _aug_w(wT, bias):
                    a = np.zeros((npad, CL), np.float32)
                    a[0, :] = bias[sl].astype(np.float32, copy=False)
                    return np.ascontiguousarray(np.concatenate([wT, a], axis=0))

                wqTc = _aug_w(wqTc, bq)
                wkTc = _aug_w(wkTc, bk)
                wvTc = _aug_w(wvTc, bv)
            wpTc = np.ascontiguousarray(wp[:, sl].T.astype(np.float32, copy=False))
            in_maps.append(
                {
                    "xT": xbs,
                    "wqT": _swizzle_w(_round_fp32r(wqTc)),
                    "wkT": _swizzle_w(_round_fp32r(wkTc)),
                    "wvT": _swizzle_w(_round_fp32r(wvTc)),
                    "wpT": _swizzle_w(_round_fp32r(wpTc)),
                    "ropeC": ropeC,
                    "ropeS": ropeS,
                    "mask01": mask01,
                }
            )
    return in_maps


def _gather(results, bp):
    out = np.empty((B, T, C), dtype=np.float32)
    bp32 = np.asarray(bp, dtype=np.float32)
    for b in range(B):
        out[b] = results[2 * b]["out"] + results[2 * b + 1]["out"] + bp32
    return out


def run(x, wq, bq, wk, bk, wv, bv, wp, bp, trace=False, **kw):
    """Build/compile (cached), run on 8 cores, gather. Returns (out, results)."""
    arrs = [np.asarray(a) for a in (x, wq, bq, wk, bk, wv, bv, wp, bp)]
    x, wq, bq, wk, bk, wv, bv, wp, bp = arrs
    with_bias = bool(np.any(bq) or np.any(bk) or np.any(bv))
    nc = _get_nc(with_bias)
    in_maps = _make_in_maps(x, wq, bq, wk, bk, wv, bv, wp, with_bias)
    res = run_bass_kernel_spmd(nc, in_maps, list(range(NCORES)), trace=trace, **kw)
    return _gather(res.results, bp), res


def kernel(x, wq, bq, wk, bk, wv, bv, wp, bp):
    out, _ = run(x, wq, bq, wk, bk, wv, bv, wp, bp)
    return out


# revision 12
# speedup vs baseline: 1.4628x; 1.0256x over previous
"""Causal self-attention (B=4, T=2048, C=1024, H=16, rope) on 8 trn2 cores.

Sharding: data-parallel over B (4) x tensor-parallel over heads (2 groups of
8 heads). Core (b, g) computes its batch's Q/K/V for its 8 heads, the full
causal attention for those heads, and a partial output projection
(y_heads @ wp_cols.T). Host sums the two head-group partials per batch and
adds the output bias.

Device layout notes:
  - Q^T/K^T are kept as [c_out, t] tiles (partition = head-dim, 2 heads per
    128-partition tile) so QK^T needs no transposes; scores are computed as
    S^T[j, i] tiles (partition = key pos, free = query pos).
  - RoPE is applied as qt = ps*C + swap(ps)*S' where swap() is realized by
    32-row SBUF->SBUF DMAs on the sync queue and C/S' are host-precomputed
    tables.
  - Softmax denominators come free from an extra all-ones column appended to
    V (row 64 of the O^T accumulation); no max-subtraction is needed because
    the logits are bounded for this problem scale.
  - The causal mask for diagonal 128x128 blocks is applied AFTER the exp as
    a 0/1 tensor_mul on GpSimd (saves the PE mask-matmuls; exp of unmasked
    entries is bounded ~e^3 so no overflow).
  - The denominator chain is: DVE reciprocal straight off the O^T PSUM row,
    then a partition-broadcast DMA (stride-0 read) on the gpsimd queue, then
    a GpSimd norm-multiply -- no pack/unpack round trips.
  - HBM tensors are laid out so every initial DMA reads fully-contiguous
    per-partition lines (8-16 KiB descriptors, full HBM rate).
  - Matmul operands are bf16 (full-rate PE path); accumulation stays fp32 in
    PSUM. A short PE pre-warm covers the initial DMA window so the HAM clock
    gate is at 8/8 when real work arrives.
"""

import sys

if "/opt/trn_rl_repo" not in sys.path:
    sys.path.insert(0, "/opt/trn_rl_repo")

from contextlib import ExitStack

import numpy as np

import concourse.bass as bass
import concourse.mybir as mybir
from concourse import bacc
from concourse.bass_utils import run_bass_kernel_spmd
from concourse.tile import TileContext

B, T, C = 4, 2048, 1024
H = 16
D = 64
NCORES = 8
CL = C // 2  # per-core c_out (8 heads * 64)
HL = 8  # local heads
F = mybir.dt.float32
FR = mybir.dt.bfloat16  # matmul operand dtype

_NC_CACHE = {}


def _build_nc(with_bias: bool):
    KC = 9 if with_bias else 8  # c_in chunks of 128 (one extra for bias row)
    nc = bacc.Bacc("TRN2", debug=False, num_devices=NCORES)

    # Host-side pre-swizzled layouts: fully-contiguous per-partition lines so
    # each initial DMA moves 8-16KB per partition in one descriptor.
    xT = nc.declare_dram_parameter("xT", [128, 2, KC * 1024], FR, isOutput=False).ap()
    wkqT = nc.declare_dram_parameter(
        "wkqT", [128, 2 * KC * CL], FR, isOutput=False
    ).ap()
    wvpT = nc.declare_dram_parameter(
        "wvpT", [128, KC * CL + 4 * C], FR, isOutput=False
    ).ap()
    ropeC = nc.declare_dram_parameter("ropeC", [128, T], FR, isOutput=False).ap()
    ropeS = nc.declare_dram_parameter("ropeS", [128, T], FR, isOutput=False).ap()
    mask01 = nc.declare_dram_parameter("mask01", [128, 128], FR, isOutput=False).ap()
    out = nc.declare_dram_parameter("out", [T, C], F, isOutput=True).ap()

    EXP = mybir.ActivationFunctionType.Exp
    scale = 1.0 / float(np.sqrt(D))

    with TileContext(nc) as tc:
        with ExitStack() as ctx:
            # pools that live across both phases
            qk_pool = ctx.enter_context(tc.tile_pool(name="qk", bufs=1))
            v_pool = ctx.enter_context(tc.tile_pool(name="v", bufs=1))
            c2 = ctx.enter_context(tc.tile_pool(name="c2", bufs=1))

            qt_sb = [
                qk_pool.tile([128, T], FR, tag=f"qt{m}", name=f"qt{m}")
                for m in range(4)
            ]
            kt_sb = [
                qk_pool.tile([128, T], FR, tag=f"kt{m}", name=f"kt{m}")
                for m in range(4)
            ]
            # all 16 V tiles in one buffer: [t-block, head, D+1]; col D is the
            # all-ones denominator column
            vaug = v_pool.tile([128, 16, HL, D + 1], FR, tag="va", name="va")

            mk_sb = c2.tile([128, 128], FR, tag="mk", name="mk")

            # ---------------- phase 1: QKV projections + rope ----------------
            with ExitStack() as p1:
                wpool = p1.enter_context(tc.tile_pool(name="w", bufs=1))
                xpool = p1.enter_context(tc.tile_pool(name="x", bufs=1))
                rpool = p1.enter_context(tc.tile_pool(name="rope", bufs=1))
                tpool = p1.enter_context(tc.tile_pool(name="t1", bufs=3))
                ps1 = p1.enter_context(tc.tile_pool(name="ps1", bufs=4, space="PSUM"))
                wrm = p1.enter_context(tc.tile_pool(name="wrm", bufs=1, space="PSUM"))

                # PE pre-warm: keep the HAM clock gate at 8/8 while the input
                # DMAs land, so the first real matmuls run at 2.4 GHz.
                warm_sb = tpool.tile([128, 512], FR, tag="warm", name="warm")
                nc.vector.memset(warm_sb, 0.0)
                warm_ps = wrm.tile([128, 512], F, tag="wps", name="wps")
                for _ in range(24):
                    nc.tensor.matmul(
                        warm_ps,
                        lhsT=warm_sb[:, 0:128],
                        rhs=warm_sb,
                        start=True,
                        stop=True,
                        skip_group_check=True,
                    )

                # initial loads: x halves on the sync queue, weights on the
                # scalar queue -- all transfers are fully contiguous per
                # partition so they run at HBM line rate. The first K matmul
                # needs only wk + x half 0 (~3MB).
                x_sb = xpool.tile([128, 2, KC * 1024], FR, tag="x", name="x")
                wkq_sb = wpool.tile([128, 2 * KC * CL], FR, tag="wkq", name="wkq")
                wvp_sb = c2.tile(
                    [128, KC * CL + 4 * C], FR, tag="wvp", name="wvp"
                )
                wk_sb = wkq_sb[:, 0 : KC * CL]
                wq_sb = wkq_sb[:, KC * CL : 2 * KC * CL]
                wv_sb = wvp_sb[:, 0 : KC * CL]
                nc.scalar.dma_start(out=wkq_sb, in_=wkqT)
                nc.sync.dma_start(out=x_sb[:, 0, :], in_=xT[:, 0, :])
                nc.scalar.dma_start(out=x_sb[:, 1, :], in_=xT[:, 1, :])
                nc.scalar.dma_start(out=wvp_sb, in_=wvpT)

                rc_sb = rpool.tile([128, T], FR, tag="rc", name="rc")
                rs_sb = rpool.tile([128, T], FR, tag="rs", name="rs")
                nc.gpsimd.dma_start(out=rc_sb, in_=ropeC)
                nc.gpsimd.dma_start(out=rs_sb, in_=ropeS)
                nc.gpsimd.dma_start(out=mk_sb, in_=mask01)

                # ones column of vaug (denominator trick)
                nc.vector.memset(vaug[:, :, :, D : D + 1], 1.0)

                def x_rhs(k, lo, hi):
                    """x^T slice [128, hi-lo] for contraction chunk k, T cols
                    [lo, hi) (must stay within one T-half)."""
                    h = lo // 1024
                    o = lo - 1024 * h
                    return x_sb[:, h, 1024 * k + o : 1024 * k + o + (hi - lo)]

                def emit_kq(m, t):
                    # K and Q tiles ([c_out, t] layout) + rope, paired per
                    # (m, t) so each 32-row rope swap DMA moves both
                    # projections at once (2KB per-partition lines). Swap DMAs
                    # ride the sync queue, which is idle after the x load.
                    kq_ps = []
                    for wsb in (wk_sb, wq_sb):
                        ps = ps1.tile([128, 512], F, tag="ps", name="ps")
                        for k in range(KC):
                            nc.tensor.matmul(
                                ps,
                                lhsT=wsb[:, CL * k + 128 * m : CL * k + 128 * (m + 1)],
                                rhs=x_rhs(k, 512 * t, 512 * (t + 1)),
                                start=(k == 0),
                                stop=(k == KC - 1),
                            )
                        kq_ps.append(ps)
                    cp = tpool.tile([128, 1024], FR, tag="cp", name="cp")
                    nc.scalar.copy(cp[:, 0:512], kq_ps[0])
                    nc.scalar.copy(cp[:, 512:1024], kq_ps[1])
                    sw = tpool.tile([128, 1024], FR, tag="sw", name="sw")
                    for a, b in ((0, 32), (32, 0), (64, 96), (96, 64)):
                        nc.sync.dma_start(out=sw[a : a + 32, :], in_=cp[b : b + 32, :])
                    ts = slice(512 * t, 512 * (t + 1))
                    t1 = tpool.tile([128, 1024], FR, tag="t1", name="t1")
                    nc.vector.tensor_mul(t1[:, 0:512], kq_ps[0], rc_sb[:, ts])
                    nc.vector.tensor_mul(t1[:, 512:1024], kq_ps[1], rc_sb[:, ts])
                    t2 = tpool.tile([128, 1024], FR, tag="t2", name="t2")
                    nc.gpsimd.tensor_mul(t2[:, 0:512], sw[:, 0:512], rs_sb[:, ts])
                    nc.gpsimd.tensor_mul(t2[:, 512:1024], sw[:, 512:1024], rs_sb[:, ts])
                    nc.vector.tensor_add(kt_sb[m][:, ts], t1[:, 0:512], t2[:, 0:512])
                    nc.vector.tensor_add(
                        qt_sb[m][:, ts], t1[:, 512:1024], t2[:, 512:1024]
                    )

                def emit_v(jj):
                    # V tile (natural [t, c_out] layout) -> vaug, extraction on
                    # the scalar engine
                    ps = ps1.tile([128, 512], F, tag="ps", name="ps")
                    for k in range(KC):
                        nc.tensor.matmul(
                            ps,
                            lhsT=x_rhs(k, 128 * jj, 128 * (jj + 1)),
                            rhs=wv_sb[:, CL * k : CL * (k + 1)],
                            start=(k == 0),
                            stop=(k == KC - 1),
                        )
                    nc.scalar.copy(
                        out=vaug[:, jj, :, 0:D],
                        in_=ps.rearrange("p (h d) -> p h d", h=HL),
                    )

                # interleave: K/Q for the first T-half, then V tiles of that
                # half (so attention for early ci can start while the second
                # half projects), then the rest.
                for m in range(4):
                    for t in (0, 1):
                        emit_kq(m, t)
                for jj in range(8):
                    emit_v(jj)
                for m in range(4):
                    for t in (2, 3):
                        emit_kq(m, t)
                for jj in range(8, 16):
                    emit_v(jj)

            wvp_view = wvp_sb
            # ---------------- phase 2: attention + output projection ---------
            ppool = ctx.enter_context(tc.tile_pool(name="pt", bufs=4))
            yrawp = ctx.enter_context(tc.tile_pool(name="yraw", bufs=4))
            ynp = ctx.enter_context(tc.tile_pool(name="yn", bufs=9))
            osbp = ctx.enter_context(tc.tile_pool(name="osb", bufs=3))
            dpool = ctx.enter_context(tc.tile_pool(name="dd", bufs=2))
            bcpool = ctx.enter_context(tc.tile_pool(name="bc", bufs=3))
            spool = ctx.enter_context(tc.tile_pool(name="sps", bufs=2, space="PSUM"))
            opool = ctx.enter_context(tc.tile_pool(name="ops", bufs=2, space="PSUM"))

            wp_sb = wvp_view[:, KC * CL :]
            ones1 = c2.tile([128, 64], F, tag="on1", name="on1")
            nc.vector.memset(ones1[0:1, :], 1.0)

            def emit_outproj_chunk(ci, yn, chunk):
                for g in (2 * chunk, 2 * chunk + 1):
                    tt, cc = g % 4, g // 4
                    pr = opool.tile([128, 512], F, tag="o", name="pr")
                    for p in range(4):
                        nc.tensor.matmul(
                            pr,
                            lhsT=yn[p][:, 128 * tt : 128 * (tt + 1)],
                            rhs=wp_sb[:, 1024 * p + 512 * cc : 1024 * p + 512 * (cc + 1)],
                            start=(p == 0),
                            stop=(p == 3),
                        )
                    osb = osbp.tile([128, 512], F, tag="osb", name="osb")
                    nc.vector.tensor_copy(osb, pr)
                    nc.sync.dma_start(
                        out=out[
                            512 * ci + 128 * tt : 512 * ci + 128 * (tt + 1),
                            512 * cc : 512 * (cc + 1),
                        ],
                        in_=osb,
                    )

            outq = []  # (yn_list, ci, next_chunk) FIFO of outproj work
            dve_q = []  # deferred boundary ops, drained 2 per tj iter
            pending_norm = None
            for ci in range(4):
                yn = []
                for p in range(4):
                    # bound the deferral window to one boundary's worth so
                    # pool reuse (WAR) tracking stays consistent with the
                    # actual emission order
                    while len(dve_q) > 7:
                        dve_q.pop(0)()
                    o_ps = opool.tile([128, 1024], F, tag="o", name="o")
                    ntj = 4 * ci + 4
                    for tj in range(ntj):
                        kk = tj - 4 * ci
                        off = 128 * max(kk, 0)
                        s_ps = spool.tile([128, 1024], F, tag="s", name="s")
                        for h in range(2):
                            nc.tensor.matmul(
                                s_ps[:, 512 * h + off : 512 * h + 512],
                                lhsT=kt_sb[p][
                                    64 * h : 64 * h + 64,
                                    128 * tj : 128 * (tj + 1),
                                ],
                                rhs=qt_sb[p][
                                    64 * h : 64 * h + 64,
                                    512 * ci + off : 512 * (ci + 1),
                                ],
                                start=True,
                                stop=True,
                                tile_position=(64 * h, 0),
                                skip_group_check=True,
                            )
                        pt = ppool.tile([128, 1024], FR, tag="pt", name="pt")
                        if kk < 0:
                            nc.scalar.activation(pt, s_ps, EXP, scale=scale)
                        else:
                            s_v = s_ps.rearrange("q (h n) -> q h n", h=2)[:, :, off:]
                            p_v = pt.rearrange("q (h n) -> q h n", h=2)[:, :, off:]
                            nc.scalar.activation(p_v, s_v, EXP, scale=scale)
                            # causal mask for the diagonal 128x128 block of
                            # both heads in ONE DVE op: 3D view [q, h, 128]
                            # of pt against a 0-stride broadcast view of the
                            # 0/1 upper-tri mask
                            pm = pt.rearrange("q (h n) -> q h n", h=2)[
                                :, :, off : off + 128
                            ]
                            mk2 = bass.AP(
                                tensor=mk_sb.tensor,
                                offset=mk_sb.offset,
                                ap=[list(mk_sb.ap[0]), [0, 2], list(mk_sb.ap[1])],
                            )
                            nc.vector.tensor_mul(pm, pm, mk2)
                        for h in range(2):
                            nc.tensor.matmul(
                                o_ps[0 : D + 1, 512 * h + off : 512 * h + 512],
                                lhsT=vaug[:, tj, 2 * p + h, :],
                                rhs=pt[:, 512 * h + off : 512 * h + 512],
                                start=(tj == 0),
                                stop=(tj == ntj - 1),
                                skip_group_check=True,
                            )
                        for _ in range(3 if ci == 3 else 2):
                            if dve_q:
                                dve_q.pop(0)()
                    # epilogue, ordered so the recip -> bc-broadcast ->
                    # norm-mul chain is always a full boundary ahead of its
                    # consumer and the PE never waits on it:
                    #   1. reciprocal of the denominator rows straight out of
                    #      PSUM (row 64 of each head's O^T accumulation)
                    #   2. compact both heads' O (shifted DVE copies, no DMA)
                    #   3. partition-broadcast of the reciprocals (stride-0
                    #      gpsimd DMA, feeds NEXT boundary's deferred norm)
                    #   4. deferred norm for the previous pair (GpSimd mul)
                    #   5. one outproj chunk-call from the FIFO (two
                    #      boundaries behind its ci, so its yn lhsT is ready)
                    yraw = yrawp.tile([128, 512], F, tag="yraw", name="yraw")
                    d_sb = dpool.tile([128, 2048], F, tag="D", name="D")
                    # denominator rows PSUM -> SBUF (plain shifted DVE copies;
                    # the custom-DVE reciprocal can't read PSUM), then ONE
                    # streaming-rate approx reciprocal over both heads' rows
                    dve_q.append(
                        lambda d=d_sb, o=o_ps: nc.vector.tensor_copy(
                            d[0:1, 0:1024], o[64:65, :]
                        )
                    )
                    dve_q.append(
                        lambda y=yraw, o=o_ps: nc.vector.tensor_copy(
                            y[0:64, :], o[0:64, 0:512]
                        )
                    )
                    dve_q.append(
                        lambda y=yraw, o=o_ps: nc.vector.tensor_copy(
                            y[64:128, :], o[0:64, 512:1024]
                        )
                    )
                    dve_q.append(
                        lambda d=d_sb: nc.vector.reciprocal_approx_fast(
                            out=d[0:1, 1024:2048], in_=d[0:1, 0:1024]
                        )
                    )
                    bc = bcpool.tile([128, 512], F, tag="bc", name="bc")

                    def mk_bc(d_sb, bc, h):
                        def f():
                            sl = d_sb[0:1, 1024 + 512 * h : 1024 + 512 * h + 512]
                            bsrc = bass.AP(
                                tensor=sl.tensor,
                                offset=sl.offset,
                                ap=[list(sl.ap[0]), [0, 64], [1, 512]],
                            )
                            nc.gpsimd.dma_start(
                                out=bc[64 * h : 64 * h + 64, :], in_=bsrc
                            )

                        return f

                    if not (ci == 3 and p == 3):
                        for h in range(2):
                            dve_q.append(mk_bc(d_sb, bc, h))

                    def mk_norm(pyn, pyraw, pbc):
                        def f():
                            pynorm = ynp.tile([128, 512], FR, tag="yn", name="yn")
                            nc.gpsimd.tensor_mul(pynorm, pyraw, pbc)
                            pyn.append(pynorm)

                        return f

                    if pending_norm is not None:
                        dve_q.append(mk_norm(*pending_norm))
                        pending_norm = None
                    if ci == 3 and p == 3:
                        # final boundary: skip the broadcast DMA; broadcast
                        # the reciprocals with two K=1 PE matmuls into a free
                        # score bank and normalize straight out of PSUM
                        def tail_norm(pyn=yn, pyraw=yraw, d=d_sb):
                            bc_ps = spool.tile([128, 1024], F, tag="s", name="bcp")
                            for h in range(2):
                                nc.tensor.matmul(
                                    bc_ps[0:64, 512 * h : 512 * h + 512],
                                    lhsT=ones1[0:1, :],
                                    rhs=d[0:1, 1024 + 512 * h : 1536 + 512 * h],
                                    start=True,
                                    stop=True,
                                    skip_group_check=True,
                                )
                            pynorm = ynp.tile([128, 512], FR, tag="yn", name="yn")
                            nc.vector.tensor_mul(
                                pynorm[0:64, :], pyraw[0:64, :], bc_ps[0:64, 0:512]
                            )
                            nc.vector.tensor_mul(
                                pynorm[64:128, :],
                                pyraw[64:128, :],
                                bc_ps[0:64, 512:1024],
                            )
                            pyn.append(pynorm)

                        dve_q.append(tail_norm)
                    elif ci == 3:
                        dve_q.append(mk_norm(yn, yraw, bc))
                    else:
                        pending_norm = (yn, yraw, bc)
                    ncalls = 2 if len(outq) > 1 else 1
                    for _ in range(ncalls):
                        if not (
                            outq
                            and outq[0][1] <= ci - 1
                            and (p >= 1 or outq[0][1] <= ci - 2)
                        ):
                            break
                        pyn_l, pci, chunk = outq[0]
                        while len(pyn_l) < 4 and dve_q:
                            dve_q.pop(0)()
                        emit_outproj_chunk(pci, pyn_l, chunk)
                        if chunk == 3:
                            outq.pop(0)
                        else:
                            outq[0][2] = chunk + 1

                outq.append([yn, ci, 0])
            for pyn_l, pci, chunk in [
                (q[0], q[1], c) for q in outq for c in range(q[2], 4)
            ]:
                while len(pyn_l) < 4 and dve_q:
                    dve_q.pop(0)()
                emit_outproj_chunk(pci, pyn_l, chunk)
            while dve_q:
                dve_q.pop(0)()

    nc.compile()
    return nc


def _get_nc(with_bias: bool):
    if with_bias not in _NC_CACHE:
        _NC_CACHE[with_bias] = _build_nc(with_bias)
    return _NC_CACHE[with_bias]


def _rope_tables():
    half = D // 2
    i = np.arange(half, dtype=np.float32)
    expo = (2.0 * i / np.float32(D)).astype(np.float32)
    alpha = (1.0 / (np.float32(10000.0) ** expo)).astype(np.float32)
    ang = (np.arange(T, dtype=np.float32)[:, None] * alpha[None, :]).astype(np.float32)
    cosv = np.cos(ang).astype(np.float32).T  # [32, T]
    sinv = np.sin(ang).astype(np.float32).T
    c64 = np.concatenate([cosv, cosv], axis=0)  # [64, T]
    s64 = np.concatenate([-sinv, sinv], axis=0)
    ropeC = np.ascontiguousarray(np.concatenate([c64, c64], axis=0))  # [128, T]
    ropeS = np.ascontiguousarray(np.concatenate([s64, s64], axis=0))
    import ml_dtypes

    return ropeC.astype(ml_dtypes.bfloat16), ropeS.astype(ml_dtypes.bfloat16)


import ml_dtypes


def _round_fp32r(a):
    """Cast host data to the matmul operand dtype (bf16)."""
    return np.ascontiguousarray(
        np.asarray(a, dtype=np.float32).astype(ml_dtypes.bfloat16)
    )


def _swizzle_w(wT):
    """[KC*128, M] -> [128, KC*M] with fully-contiguous per-partition lines."""
    kc = wT.shape[0] // 128
    m = wT.shape[1]
    return np.ascontiguousarray(
        wT.reshape(kc, 128, m).transpose(1, 0, 2).reshape(128, kc * m)
    )


def _swizzle_x(xb):
    """[KC*128, T] -> [128, 2, KC*1024]: partition-major, T-half-major, then
    (chunk, within-half-col) contiguous."""
    kc = xb.shape[0] // 128
    return np.ascontiguousarray(
        xb.reshape(kc, 128, 2, 1024).transpose(1, 2, 0, 3).reshape(128, 2, kc * 1024)
    )


def _make_in_maps(x, wq, bq, wk, bk, wv, bv, wp, with_bias):
    ropeC, ropeS = _rope_tables()
    # 0/1 causal keep-mask for the diagonal 128x128 block: 1 where j <= i
    # (keep: key j, query i), 0 where masked
    mask01 = np.triu(np.ones((128, 128), np.float32)).astype(ml_dtypes.bfloat16)
    in_maps = []
    for b in range(B):
        xb = np.ascontiguousarray(x[b].T.astype(np.float32, copy=False))  # [C, T]
        if with_bias:
            aug = np.zeros((9 * 128 - C, T), np.float32)
            aug[0, :] = 1.0
            xb = np.concatenate([xb, aug], axis=0)
        xbs = _swizzle_x(_round_fp32r(xb))
        for g in range(2):
            sl = slice(g * CL, (g + 1) * CL)
            wqTc = np.ascontiguousarray(wq[sl, :].T.astype(np.float32, copy=False))
            wkTc = np.ascontiguousarray(wk[sl, :].T.astype(np.float32, copy=False))
            wvTc = np.ascontiguousarray(wv[sl, :].T.astype(np.float32, copy=False))
            if with_bias:
                npad = 9 * 128 - C

                def _aug_w(wT, bias):
                    a = np.zeros((npad, CL), np.float32)
                    a[0, :] = bias[sl].astype(np.float32, copy=False)
                    return np.ascontiguousarray(np.concatenate([wT, a], axis=0))

                wqTc = _aug_w(wqTc, bq)
                wkTc = _aug_w(wkTc, bk)
                wvTc = _aug_w(wvTc, bv)
            wpTc = np.ascontiguousarray(wp[:, sl].T.astype(np.float32, copy=False))
            wkq = np.concatenate(
                [_swizzle_w(_round_fp32r(wkTc)), _swizzle_w(_round_fp32r(wqTc))],
                axis=1,
            )
            wvp = np.concatenate(
                [_swizzle_w(_round_fp32r(wvTc)), _swizzle_w(_round_fp32r(wpTc))],
                axis=1,
            )
            in_maps.append(
                {
                    "xT": xbs,
                    "wkqT": np.ascontiguousarray(wkq),
                    "wvpT": np.ascontiguousarray(wvp),
                    "ropeC": ropeC,
                    "ropeS": ropeS,
                    "mask01": mask01,
                }
            )
    return in_maps


def _gather(results, bp):
    out = np.empty((B, T, C), dtype=np.float32)
    bp32 = np.asarray(bp, dtype=np.float32)
    for b in range(B):
        out[b] = results[2 * b]["out"] + results[2 * b + 1]["out"] + bp32
    return out


def run(x, wq, bq, wk, bk, wv, bv, wp, bp, trace=False, **kw):
    """Build/compile (cached), run on 8 cores, gather. Returns (out, results)."""
    arrs = [np.asarray(a) for a in (x, wq, bq, wk, bk, wv, bv, wp, bp)]
    x, wq, bq, wk, bk, wv, bv, wp, bp = arrs
    with_bias = bool(np.any(bq) or np.any(bk) or np.any(bv))
    nc = _get_nc(with_bias)
    in_maps = _make_in_maps(x, wq, bq, wk, bk, wv, bv, wp, with_bias)
    res = run_bass_kernel_spmd(nc, in_maps, list(range(NCORES)), trace=trace, **kw)
    return _gather(res.results, bp), res


def kernel(x, wq, bq, wk, bk, wv, bv, wp, bp):
    out, _ = run(x, wq, bq, wk, bk, wv, bv, wp, bp)
    return out
